# revision 38
# baseline (speedup 1.0000x reference)
"""DyHeadBlock Trainium2 kernel: 8-core row-sharded Bass/Tile implementation.

kernel(**inputs) -> (out0, out1, out2) matching reference.reference(**inputs).
Self-contained: hardcodes all shapes/sharding.
"""
import numpy as np

B, C = 2, 256
HW_L = [(80, 80), (40, 40), (20, 20)]
NCOR = 8
EPS = 1e-5

_CACHE = {}


def _shards(H):
    return [c * H // NCOR for c in range(NCOR)], [(c + 1) * H // NCOR for c in range(NCOR)]


class _C:
    pass


def build_cfg():
    cfg = _C()
    cfg.fine, cfg.coarse = [], []
    for l in range(3):
        H, W = HW_L[l]
        s, e = _shards(H)
        span = max(e[c] - s[c] for c in range(NCOR))
        win = [max(0, min(s[c], H - span)) for c in range(NCOR)]
        cfg.fine.append(dict(s=s, e=e, span=span, win=win))
    for l in range(2):
        Hf = HW_L[l][0]
        Hc = HW_L[l + 1][0]
        s, e = _shards(Hc)
        lo, hi = [], []
        for c in range(NCOR):
            f = cfg.fine[l]
            fr = np.arange(f['win'][c], f['win'][c] + f['span'])
            ys = fr * (Hc - 1) / (Hf - 1)
            lo.append(min(int(np.floor(ys.min())), s[c]))
            hi.append(max(min(int(np.floor(ys.max())) + 1, Hc - 1), e[c] - 1))
        span = max(h - l0 + 1 for h, l0 in zip(hi, lo))
        win = [max(0, min(lo[c], Hc - span)) for c in range(NCOR)]
        cfg.coarse.append(dict(s=s, e=e, span=span, win=win))
    # tasks: (level, branch) -> output grid shard + input info
    cfg.tinfo = []
    for l in range(3):
        defs = [(l, 'mid', l, 1, 0)]
        if l > 0:
            defs.append((l, 'low', l - 1, 2, 1))
        if l < 2:
            defs.append((l, 'hi', l + 1, 1, 2))
        for (ll, br, sl, st, wi) in defs:
            if br == 'hi':
                Hg, Wg = HW_L[ll + 1]
                span = cfg.coarse[ll]['span']
            else:
                Hg, Wg = HW_L[ll]
                span = cfg.fine[ll]['span']
            Hin, Win = HW_L[sl]
            N = span * Wg
            Npad = -(-N // 128) * 128
            cfg.tinfo.append(dict(l=ll, br=br, sl=sl, st=st, wi=wi, span=span,
                                  W=Wg, Hg=Hg, Hin=Hin, Win=Win, N=N, Npad=Npad))
    cfg.NT = 7
    cfg.NP = max(t['Npad'] for t in cfg.tinfo)
    # gather slabs (channel-last source rows per task)
    cfg.slab = []
    for ti, t in enumerate(cfg.tinfo):
        Hin = t['Hin']
        lo, hi = [], []
        for c in range(NCOR):
            w0 = (cfg.coarse[t['l']] if t['br'] == 'hi' else cfg.fine[t['l']])['win'][c]
            lo.append(max(0, w0 * t['st'] - 6))
            hi.append(min(Hin - 1, (w0 + t['span'] - 1) * t['st'] + 6))
        span = max(h - l0 + 1 for h, l0 in zip(hi, lo))
        win = [max(0, min(lo[c], Hin - span)) for c in range(NCOR)]
        cfg.slab.append(dict(span=span, win=win))
    cfg.resize = []
    for l in range(2):
        f = cfg.fine[l]
        Hc, Wc = HW_L[l + 1]
        Hf, Wf = HW_L[l]
        N = f['span'] * Wf
        cfg.resize.append(dict(l=l, span=f['span'], Wf=Wf, Hf=Hf, Hc=Hc, Wc=Wc,
                               N=N, Npad=-(-N // 128) * 128))
    # level -> task ids
    cfg.ltasks = [[ti for ti, t in enumerate(cfg.tinfo) if t['l'] == l] for l in range(3)]
    return cfg


CFG = build_cfg()


def _bf16():
    import ml_dtypes
    return ml_dtypes.bfloat16


# ===========================================================================
# host prep
# ===========================================================================

def host_prep(inputs, cfg):
    xs = [np.asarray(inputs['x0'], np.float32), np.asarray(inputs['x1'], np.float32),
          np.asarray(inputs['x2'], np.float32)]
    off_w = np.asarray(inputs['off_w'], np.float32)
    off_b = np.asarray(inputs['off_b'], np.float32)
    shared = {}
    # permute offset conv out-channels: [dy0..8, dx0..8, mask0..8]
    perm = np.concatenate([np.arange(0, 18, 2), np.arange(1, 18, 2), np.arange(18, 27)])
    off_w = off_w[perm]
    off_b = off_b[perm]
    offw_t = np.zeros((9, 2, 128, 32), np.float32)
    for k in range(9):
        w = off_w[:, :, k // 3, k % 3]
        offw_t[k, 0, :, :27] = w[:, :128].T
        offw_t[k, 1, :, :27] = w[:, 128:].T
    shared['offw_t'] = offw_t.reshape(9 * 2 * 128, 32).astype(_bf16())
    for wi, wn in enumerate(['w_mid', 'w_low', 'w_high']):
        w = np.asarray(inputs[wn], np.float32).reshape(C, C, 9)
        tt = np.zeros((18, 128, 256), np.float32)
        for k in range(9):
            for ch in range(2):
                tt[k * 2 + ch] = w[:, ch * 128:(ch + 1) * 128, k].T
        shared[f'wd{wi}'] = tt.reshape(18 * 128, 256).astype(_bf16())
    shared['gamma'] = np.stack([np.asarray(inputs[g], np.float32) for g in
                                ['g_mid', 'g_low', 'g_high']])       # [3, 256]
    shared['beta'] = np.stack([np.asarray(inputs[g], np.float32) for g in
                               ['be_mid', 'be_low', 'be_high']])
    shared['swcol'] = np.asarray(inputs['scale_w'], np.float32).reshape(2, 128).T.copy()  # [128, 2]
    shared['swb'] = np.full((14, 1), float(np.asarray(inputs['scale_b']).reshape(())), np.float32)
    shared['dy1t'] = np.asarray(inputs['dy1_w'], np.float32).reshape(64, 2, 128).transpose(2, 1, 0).reshape(128, 128).copy()
    shared['dy1b'] = np.asarray(inputs['dy1_b'], np.float32).reshape(64, 1)
    shared['dy2t'] = np.asarray(inputs['dy2_w'], np.float32).T.copy()  # [64, 1024]
    shared['dy2b'] = np.asarray(inputs['dy2_b'], np.float32).reshape(8, 128).T.copy()  # [128, 8]
    indt = np.zeros((128, 8), np.float32)
    indt[np.arange(128), np.arange(128) // 16] = 1.0
    shared['indt'] = indt
    ind8 = np.zeros((8, 128), np.float32)
    for g in range(8):
        ind8[g, g * 16:(g + 1) * 16] = 1.0
    shared['ind8'] = ind8
    shared['ident'] = np.eye(128, dtype=np.float32)
    shared['ones14'] = np.ones((14, 128), np.float32)
    msel = np.zeros((128, 8, 128), np.float32)
    for u in range(128):
        for q in range(128):
            for j in range(8):
                if u == 16 * j + (q % 16):
                    msel[u, j, q] = 1.0
    shared['msel'] = msel.reshape(128, 8 * 128)
    shared['off_b'] = np.pad(off_b, (0, 5)).reshape(32, 1)

    in_maps = []
    for c in range(NCOR):
        m = dict(shared)
        # cm slabs: fine windows (+1 row halo each side), zero-padded cols
        for l in range(3):
            H, W = HW_L[l]
            fw = cfg.fine[l]['win'][c]
            rows = cfg.fine[l]['span'] + 2
            slab = np.zeros((B, C, rows, W + 2), np.float32)
            r0 = max(0, fw - 1)
            r1 = min(H, fw - 1 + rows)
            slab[:, :, r0 - (fw - 1):r1 - (fw - 1), 1:W + 1] = xs[l][:, :, r0:r1, :]
            m[f'xcm{l}'] = slab.reshape(B, C, rows * (W + 2)).astype(_bf16())
        # hi-grid cm slabs: rows 2*cw-1 .. 2*cw+2*span_c-1
        for l in range(2):
            H, W = HW_L[l]
            cw = cfg.coarse[l]['win'][c]
            spc = cfg.coarse[l]['span']
            rows = 2 * spc + 1
            slab = np.zeros((B, C, rows, W + 2), np.float32)
            r0 = max(0, 2 * cw - 1)
            r1 = min(H, 2 * cw - 1 + rows)
            slab[:, :, r0 - (2 * cw - 1):r1 - (2 * cw - 1), 1:W + 1] = xs[l][:, :, r0:r1, :]
            m[f'xch{l}'] = slab.reshape(B, C, rows * (W + 2)).astype(_bf16())
        for ti, t in enumerate(cfg.tinfo):
            sl = cfg.slab[ti]
            w0 = sl['win'][c]
            sub = xs[t['sl']][:, :, w0:w0 + sl['span'], :]
            m[f'xcl{ti}'] = np.ascontiguousarray(
                sub.transpose(0, 2, 3, 1).reshape(B, sl['span'] * t['Win'], C)).astype(_bf16())
        ky, kx, prow = _pipe_consts(cfg, c)
        m['ky'] = ky
        m['kx'] = kx
        m['prow'] = prow
        masks, omegas = _stats_masks(cfg, c)
        mb = np.zeros((cfg.NT, cfg.NP), np.float32)
        og = np.zeros((2, cfg.NP), np.float32)
        hi_i = 0
        for ti, t in enumerate(cfg.tinfo):
            mb[ti, :t['Npad']] = masks[ti]
            if t['br'] == 'hi':
                og[hi_i, :t['Npad']] = omegas[ti]
                hi_i += 1
        m['maskb'] = np.broadcast_to(mb[:, None, :], (cfg.NT, 128, cfg.NP)).copy()
        m['omegab'] = np.broadcast_to(og[:, None, :], (2, 128, cfg.NP)).copy()
        for l in range(2):
            idx, coef = _resize_consts(cfg, c, l)
            rz = cfg.resize[l]
            npch = rz['Npad'] // 128
            wr = np.zeros((128, npch * 32), np.int16)
            ii = np.arange(npch * 4 * 128)
            pch, blk, pl = ii // 512, (ii // 128) % 4, ii % 128
            wr[ii % 16, ii // 16] = idx[blk, pch * 128 + pl]
            wr[16:] = np.tile(wr[:16], (7, 1))
            m[f'rzidx{l}'] = wr
            cf = np.zeros((128, npch * 4), np.float32)
            for b4 in range(4):
                cf[:, b4::4] = coef[b4].reshape(npch, 128).T
            m[f'rzcoef{l}'] = cf
        in_maps.append(m)
    return in_maps


def _pipe_consts(cfg, c):
    NP = cfg.NP
    ky = np.full((128, NP), 2.25, np.float32)
    kx = np.full((128, NP), 2.25, np.float32)
    prow = np.zeros((128, 4), np.float32)
    for ri in range(126):
        ti, b, k = ri // 18, (ri // 9) % 2, ri % 9
        t = cfg.tinfo[ti]
        kh, kw = k // 3, k % 3
        w0 = (cfg.coarse[t['l']] if t['br'] == 'hi' else cfg.fine[t['l']])['win'][c]
        rows = (np.arange(t['span'])[:, None] + w0) * t['st'] - 1 + kh
        cols = np.arange(t['W'])[None, :] * t['st'] - 1 + kw
        ky[ri, :t['N']] = np.broadcast_to(rows, (t['span'], t['W'])).reshape(-1)
        kx[ri, :t['N']] = np.broadcast_to(cols, (t['span'], t['W'])).reshape(-1)
        ky[ri, t['N']:] = cfg.slab[ti]['win'][c] + 2.25   # pad -> inside slab
        prow[ri] = [t['Hin'] - 1, t['Win'] - 1, t['Win'],
                    -cfg.slab[ti]['win'][c] * t['Win']]
    prow[126:] = [1, 1, 1, 0]
    return ky, kx, prow


def _stats_masks(cfg, c):
    masks, omegas = [], []
    for ti, t in enumerate(cfg.tinfo):
        sh = cfg.coarse[t['l']] if t['br'] == 'hi' else cfg.fine[t['l']]
        w0 = sh['win'][c]
        rows = np.arange(t['span']) + w0
        ownrow = (rows >= sh['s'][c]) & (rows < sh['e'][c])
        mk = np.zeros(t['Npad'], np.float32)
        mk[:t['N']] = np.repeat(ownrow.astype(np.float32), t['W'])
        masks.append(mk)
        if t['br'] == 'hi':
            l = t['l']
            Hc, Wc = HW_L[l + 1]
            Hf, Wf = HW_L[l]
            ys = np.arange(Hf) * (Hc - 1.0) / (Hf - 1.0)
            y0 = np.floor(ys).astype(np.int64)
            y1 = np.minimum(y0 + 1, Hc - 1)
            wy = ys - y0
            xg = np.arange(Wf) * (Wc - 1.0) / (Wf - 1.0)
            x0 = np.floor(xg).astype(np.int64)
            x1 = np.minimum(x0 + 1, Wc - 1)
            wx = xg - x0
            wrow = np.zeros(Hc)
            wcol = np.zeros(Wc)
            np.add.at(wrow, y0, 1 - wy)
            np.add.at(wrow, y1, wy)
            np.add.at(wcol, x0, 1 - wx)
            np.add.at(wcol, x1, wx)
            om_full = np.outer(wrow, wcol) / (Hf * Wf)
            o = np.zeros(t['Npad'], np.float32)
            o[:t['N']] = (om_full[rows] * ownrow[:, None]).reshape(-1)
            omegas.append(o)
        else:
            omegas.append(None)
    return masks, omegas


def _resize_consts(cfg, c, l):
    rz = cfg.resize[l]
    Hc, Wc, Hf, Wf = rz['Hc'], rz['Wc'], rz['Hf'], rz['Wf']
    w0f = cfg.fine[l]['win'][c]
    w0c = cfg.coarse[l]['win'][c]
    spc = cfg.coarse[l]['span']
    rows = np.arange(rz['span']) + w0f
    ys = rows * (Hc - 1.0) / (Hf - 1.0)
    y0 = np.floor(ys).astype(np.int64)
    y1 = np.minimum(y0 + 1, Hc - 1)
    wy = (ys - y0).astype(np.float32)
    xg = np.arange(Wf) * (Wc - 1.0) / (Wf - 1.0)
    x0 = np.floor(xg).astype(np.int64)
    x1 = np.minimum(x0 + 1, Wc - 1)
    wx = (xg - x0).astype(np.float32)
    N, Npad = rz['N'], rz['Npad']
    idx = np.zeros((4, Npad), np.int32)
    coef = np.zeros((4, Npad), np.float32)
    Y0 = np.repeat(y0, Wf); Y1 = np.repeat(y1, Wf); WY = np.repeat(wy, Wf)
    X0 = np.tile(x0, rz['span']); X1 = np.tile(x1, rz['span']); WX = np.tile(wx, rz['span'])

    def rel(y):
        return np.clip(y, w0c, w0c + spc - 1) - w0c

    idx[0, :N] = rel(Y0) * Wc + X0; coef[0, :N] = (1 - WY) * (1 - WX)
    idx[1, :N] = rel(Y0) * Wc + X1; coef[1, :N] = (1 - WY) * WX
    idx[2, :N] = rel(Y1) * Wc + X0; coef[2, :N] = WY * (1 - WX)
    idx[3, :N] = rel(Y1) * Wc + X1; coef[3, :N] = WY * WX
    return idx, coef


# ===========================================================================
# device program
# ===========================================================================

def build_nc(cfg):
    import os
    STAGE = int(os.environ.get('KSTAGE', '9'))
    PART = int(os.environ.get('KPART', '9'))
    KTASK = int(os.environ.get('KTASK', '-1'))
    import concourse.bacc as bacc
    import concourse.mybir as mybir
    import concourse.tile as tile
    from contextlib import ExitStack
    dt = mybir.dt
    AT = mybir.AluOpType
    AF = mybir.ActivationFunctionType
    AX = mybir.AxisListType

    class _StopBuild(Exception):
        pass
    nc = bacc.Bacc('TRN2', target_bir_lowering=False, debug=False, num_devices=NCOR, num_swdge_queues=4)
    NP = cfg.NP

    P = {}

    def par(nm, shp, d):
        P[nm] = nc.declare_dram_parameter(nm, shp, d, isOutput=False)

    par('offw_t', [9 * 2 * 128, 32], dt.bfloat16)
    for wi in range(3):
        par(f'wd{wi}', [18 * 128, 256], dt.bfloat16)
    par('gamma', [3, 256], dt.float32)
    par('beta', [3, 256], dt.float32)
    par('swcol', [128, 2], dt.float32)
    par('swb', [14, 1], dt.float32)
    par('dy1t', [128, 128], dt.float32)
    par('dy1b', [64, 1], dt.float32)
    par('dy2t', [64, 1024], dt.float32)
    par('dy2b', [128, 8], dt.float32)
    par('indt', [128, 8], dt.float32)
    par('ind8', [8, 128], dt.float32)
    par('ident', [128, 128], dt.float32)
    par('ones14', [14, 128], dt.float32)
    par('msel', [128, 8 * 128], dt.float32)
    par('off_b', [32, 1], dt.float32)
    par('ky', [128, NP], dt.float32)
    par('kx', [128, NP], dt.float32)
    par('prow', [128, 4], dt.float32)
    par('maskb', [cfg.NT, 128, NP], dt.float32)
    par('omegab', [2, 128, NP], dt.float32)
    for l in range(3):
        rows = cfg.fine[l]['span'] + 2
        par(f'xcm{l}', [B, C, rows * (HW_L[l][1] + 2)], dt.bfloat16)
    for l in range(2):
        rows = 2 * cfg.coarse[l]['span'] + 1
        par(f'xch{l}', [B, C, rows * (HW_L[l][1] + 2)], dt.bfloat16)
    for ti, t in enumerate(cfg.tinfo):
        par(f'xcl{ti}', [B, cfg.slab[ti]['span'] * t['Win'], C], dt.bfloat16)
    for l in range(2):
        npch = cfg.resize[l]['Npad'] // 128
        par(f'rzidx{l}', [128, npch * 32], dt.int16)
        par(f'rzcoef{l}', [128, npch * 4], dt.float32)
    OUT = [nc.declare_dram_parameter(f'out{l}', [B, C, cfg.fine[l]['span'] * HW_L[l][1]],
                                     dt.float32, isOutput=True) for l in range(3)]

    import contextlib
    with tile.TileContext(nc) as tc:
        with contextlib.suppress(_StopBuild), ExitStack() as ctx:
            sb = ctx.enter_context(tc.tile_pool(name='sb', bufs=1))
            sb2 = ctx.enter_context(tc.tile_pool(name='sb2', bufs=2))
            ps = ctx.enter_context(tc.tile_pool(name='ps', bufs=2, space='PSUM'))
            ps1 = ctx.enter_context(tc.tile_pool(name='ps1', bufs=1, space='PSUM'))
            dram = ctx.enter_context(tc.tile_pool(name='dram', bufs=1, space='DRAM'))

            def dma(dst, src):
                nc.sync.dma_start(dst, src)

            def tss(out, in0, s1, op0, s2=None, op1=None):
                if op1 is None:
                    nc.vector.tensor_scalar(out, in0, s1, None, op0)
                else:
                    nc.vector.tensor_scalar(out, in0, s1, s2, op0, op1)

            # ---- constants
            ident = sb.tile([128, 128], dt.float32)
            dma(ident[:], P['ident'][:])
            identb = sb.tile([128, 128], dt.bfloat16)
            nc.vector.tensor_copy(identb[:], ident[:])
            offw = sb.tile([128, 18 * 32], dt.bfloat16)
            dma(offw[:].rearrange('p (a c) -> p a c', a=18), P['offw_t'][:].rearrange('(a p) c -> p a c', p=128))
            wdt_cur = {'wi': None, 't': None}

            def get_wdt(wi):
                wt_ = sb.tile([128, 18 * 256], dt.bfloat16, tag='wdtcur', name='wdtcur')
                dma(wt_[:].rearrange('p (a c) -> p a c', a=18), P[f'wd{wi}'][:].rearrange('(a p) c -> p a c', p=128))
                return wt_
            ky = sb.tile([128, NP], dt.float32, tag='ky', name='ky')
            dma(ky[:], P['ky'][:])
            kx = sb.tile([128, NP], dt.float32, tag='kx', name='kx')
            dma(kx[:], P['kx'][:])
            prow = sb.tile([128, 4], dt.float32, tag='prow', name='prow')
            dma(prow[:], P['prow'][:])
            offb = sb.tile([32, 1], dt.float32, tag='offb', name='offb')
            dma(offb[:], P['off_b'][:])

            # ========= PHASE A: offset convs ==========
            om_f, om_h = {}, {}

            def offconv(dst, slabpar, b, span, Wout, Win, stride):
                Wp = Win + 2
                srows = (span - 1) * stride + 3
                blk = srows * Wp + 2
                slab = sb.tile([128, 2 * blk], dt.bfloat16, tag='cslab', name='cslab')
                for ct in range(2):
                    dma(slab[:, ct * blk:ct * blk + srows * Wp],
                        slabpar[b, ct * 128:(ct + 1) * 128, :])
                r = 0
                while r < span:
                    rr = min(6 if stride == 1 else 4, span - r)
                    Nch = rr * Wout
                    pt = ps1.tile([32, 512], dt.float32, tag='ompsum', name='ompsum')
                    cnt = 0
                    for k in range(9):
                        kh, kw = k // 3, k % 3
                        for ct in range(2):
                            base = ct * blk + (r * stride + kh) * Wp + kw
                            nsr = (rr - 1) * stride + 1
                            rhs = slab[:, base:base + nsr * Wp]
                            rhs = rhs.rearrange('p (r w) -> p r w', r=nsr, w=Wp)
                            if stride > 1:
                                rhs = rhs[:, ::stride, 0:(Wout - 1) * stride + 1:stride]
                            else:
                                rhs = rhs[:, :, 0:Wout]
                            nc.tensor.matmul(pt[0:27, 0:Nch],
                                             offw[:, (k * 2 + ct) * 32:(k * 2 + ct) * 32 + 27],
                                             rhs, start=(cnt == 0), stop=(cnt == 17))
                            cnt += 1
                    nc.scalar.activation(dst[0:27, r * Wout:r * Wout + Nch], pt[0:27, 0:Nch], AF.Copy)
                    r += rr

            for l in range(3):
                W = HW_L[l][1]
                for b in range(B):
                    of = sb.tile([32, cfg.fine[l]['span'] * W], dt.float32, tag=f'omf{l}{b}')
                    offconv(of, P[f'xcm{l}'], b, cfg.fine[l]['span'], W, W, 1)
                    nc.vector.tensor_scalar(of[0:27, :], of[0:27, :], offb[0:27, :], None, AT.add)
                    om_f[(l, b)] = of
                    if l < 2:
                        Wc = HW_L[l + 1][1]
                        oh = sb.tile([32, cfg.coarse[l]['span'] * Wc], dt.float32, tag=f'omh{l}{b}')
                        offconv(oh, P[f'xch{l}'], b, cfg.coarse[l]['span'], Wc, W, 2)
                        nc.vector.tensor_scalar(oh[0:27, :], oh[0:27, :], offb[0:27, :], None, AT.add)
                        om_h[(l, b)] = oh

            # ========= PHASE B: coef/idx pipeline [126, NP] ==========
            dy = sb.tile([128, NP], dt.float32)
            dx = sb.tile([128, NP], dt.float32)
            msk = sb.tile([128, NP], dt.float32)
            for t3 in (dy, dx, msk):
                nc.vector.memset(t3[:], 0.0)
            for ti, t in enumerate(cfg.tinfo):
                for b in range(B):
                    ri = (ti * 2 + b) * 9
                    src = om_h[(t['l'], b)] if t['br'] == 'hi' else om_f[(t['l'], b)]
                    dma(dy[ri:ri + 9, 0:t['N']], src[0:9, 0:t['N']])
                    dma(dx[ri:ri + 9, 0:t['N']], src[9:18, 0:t['N']])
                    dma(msk[ri:ri + 9, 0:t['N']], src[18:27, 0:t['N']])

            nc.scalar.activation(msk[:], msk[:], AF.Sigmoid)
            Hm1 = prow[:, 0:1]
            Wm1 = prow[:, 1:2]
            Wmul = prow[:, 2:3]
            shift = prow[:, 3:4]
            py = sb.tile([128, NP], dt.float32)
            px = sb.tile([128, NP], dt.float32)
            nc.vector.tensor_tensor(py[:], ky[:], dy[:], AT.add)
            nc.vector.tensor_tensor(px[:], kx[:], dx[:], AT.add)
            tmp = sb.tile([128, NP], dt.float32)
            i16 = sb.tile([128, NP], dt.int16)
            y0f = sb.tile([128, NP], dt.float32)
            x0f = sb.tile([128, NP], dt.float32)
            tss(tmp[:], py[:], 15.5, AT.add)
            nc.vector.tensor_copy(i16[:], tmp[:])
            nc.vector.tensor_copy(y0f[:], i16[:])
            tss(y0f[:], y0f[:], -16.0, AT.add)
            tss(tmp[:], px[:], 15.5, AT.add)
            nc.vector.tensor_copy(i16[:], tmp[:])
            nc.vector.tensor_copy(x0f[:], i16[:])
            tss(x0f[:], x0f[:], -16.0, AT.add)
            ly = py
            lx = px
            nc.vector.tensor_tensor(ly[:], py[:], y0f[:], AT.subtract)
            nc.vector.tensor_tensor(lx[:], px[:], x0f[:], AT.subtract)

            wgt = {}
            for (nmw, base, lfr, mlim, foldmask) in [('y', y0f, ly, Hm1, True),
                                                     ('x', x0f, lx, Wm1, False)]:
                for j in range(2):
                    v = sb.tile([128, NP], dt.float32, tag='pv', name='pv')
                    tss(v[:], base[:], float(j), AT.add)
                    g = sb.tile([128, NP], dt.float32, tag='psmall', name='pg')
                    tss(g[:], v[:], 0.0, AT.is_ge)
                    tss(v[:], v[:], mlim, AT.is_le)
                    nc.vector.tensor_tensor(g[:], g[:], v[:], AT.mult)
                    wtag = {('y', 0): 'dy', ('y', 1): 'dx', ('x', 0): 'wx0', ('x', 1): 'wx1'}[(nmw, j)]
                    w = sb.tile([128, NP], dt.float32, tag=wtag, name=wtag + 'w')
                    if j == 0:
                        tss(w[:], lfr[:], -1.0, AT.mult, 1.0, AT.add)
                    else:
                        nc.vector.tensor_copy(w[:], lfr[:])
                    nc.vector.tensor_tensor(w[:], w[:], g[:], AT.mult)
                    if foldmask:
                        nc.vector.tensor_tensor(w[:], w[:], msk[:], AT.mult)
                    wgt[(nmw, j)] = w
            ycl = {}
            xcl_ = {}
            for (nmw, base, mlim, store) in [('y', y0f, Hm1, ycl), ('x', x0f, Wm1, xcl_)]:
                for j in range(2):
                    vtag = {('y', 0): 'ky', ('y', 1): 'kx', ('x', 0): 'msk', ('x', 1): 'cx1'}[(nmw, j)]
                    v = sb.tile([128, NP], dt.float32, tag=vtag, name=vtag + 'c')
                    tss(v[:], base[:], float(j), AT.add)
                    tss(v[:], v[:], 0.0, AT.max)
                    tss(v[:], v[:], mlim, AT.min)
                    store[j] = v
            coefp = {}
            idxp = {}
            for jy in range(2):
                for jx in range(2):
                    j = jy * 2 + jx
                    cpl = sb.tile([128, NP], dt.bfloat16, tag=f'coef{j}', name=f'coef{j}')
                    nc.vector.tensor_tensor(tmp[:], wgt[('y', jy)][:], wgt[('x', jx)][:], AT.mult)
                    nc.vector.tensor_copy(cpl[:], tmp[:])
                    coefp[j] = cpl
                    ipl = sb.tile([128, NP], dt.float32, tag=f'idx{j}', name=f'idx{j}')
                    nc.vector.tensor_scalar(ipl[:], ycl[jy][:], Wmul, None, AT.mult)
                    nc.vector.tensor_tensor(ipl[:], ipl[:], xcl_[jx][:], AT.add)
                    nc.vector.tensor_scalar(ipl[:], ipl[:], shift, None, AT.add)
                    idxp[j] = ipl

            # ========= PHASE C: wrapped idx tiles (PE transposes + select) =
            wrapped = {}
            for ti, t in enumerate(cfg.tinfo):
                npch = t['Npad'] // 128
                ncols = npch * 36
                L = npch * 288
                for b in range(B):
                    ri = (ti * 2 + b) * 9
                    Tt = sb.tile([128, 252], dt.float32, tag='Tidx', name='Tidx')
                    for pch in range(npch):
                        c9i = sb2.tile([9, 4, 128], dt.float32, tag='c9i', name='c9i')
                        for j in range(4):
                            dma(c9i[:, j, :], idxp[j][ri:ri + 9, pch * 128:(pch + 1) * 128])
                        tw = ps.tile([128, 36], dt.float32, tag='po', name='tw')
                        for j in range(4):
                            nc.tensor.transpose(tw[:, j * 9:(j + 1) * 9], c9i[0:9, j, :], ident[0:9, 0:9])
                        # reorder (j,k) -> blk=k*4+j while copying
                        nc.vector.tensor_copy(
                            Tt[:, pch * 36:(pch + 1) * 36].rearrange('p (k j) -> p k j', k=9, j=4),
                            tw[:].rearrange('p (j k) -> p k j', j=4, k=9))
                    wt = sb.tile([128, L], dt.int16, tag=f'wrap{ti}{b}', name=f'wrap{ti}{b}')
                    for j in range(8):
                        msl = sb2.tile([128, 128], dt.float32, tag='msl', name='msl')
                        dma(msl[:], P['msel'][:, j * 128:(j + 1) * 128])
                        wps = ps.tile([128, 252], dt.float32, tag='po', name='wps')
                        nc.tensor.matmul(wps[:, 0:ncols], msl[:],
                                         Tt[:, 0:ncols], start=True, stop=True)
                        nc.vector.tensor_copy(
                            wt[:].rearrange('p (s j) -> p s j', j=8)[:, :, j],
                            wps[:, 0:ncols])
                    wrapped[(ti, b)] = wt

            # ========= PHASE D: gather + dcn ==========
            if STAGE < 2:
                raise _StopBuild()
            dcnout = {}
            stats = sb.tile([128, 64], dt.float32)
            nc.vector.memset(stats[:], 0.0)
            scol = {}

            def stat_col(ti, b, mh, kind):
                base = {'s1': 0, 's2': 28, 'w': 56}[kind]
                if kind == 'w':
                    hi_ix = [1, 4].index(ti)
                    return base + (hi_ix * 2 + b) * 2 + mh
                return base + (ti * 2 + b) * 2 + mh

            qn = [0]
            for ti, t in enumerate(cfg.tinfo):
                if KTASK >= 0 and ti != KTASK:
                    continue
                npch = t['Npad'] // 128
                Npos_full = t['Hg'] * t['W']
                wdtt = get_wdt(t['wi'])
                for b in range(B):
                    ri = (ti * 2 + b) * 9
                    do = dram.tile([256, t['Npad']], dt.float32, tag=f'do{ti}{b}')
                    dcnout[(ti, b)] = do
                    for pc in range(npch):
                        G = sb.tile([128, 36, 256], dt.bfloat16, tag='G', name='G')
                        for gch in range(9):
                            nc.gpsimd.dma_gather(
                                G[:, gch * 4:(gch + 1) * 4, :], P[f'xcl{ti}'][b],
                                wrapped[(ti, b)][:, pc * 288 + gch * 32: pc * 288 + (gch + 1) * 32],
                                num_idxs=512, num_idxs_reg=512, elem_size=256,
                                queue_num=qn[0] % 4)
                            qn[0] += 1
                        if PART < 2:
                            nc.gpsimd.dma_start(do[0:128, pc * 128:(pc + 1) * 128],
                                                G[:, 0, 0:128])
                            continue
                        # coef transposes -> coefT [128, 36] bf16
                        cps = ps.tile([128, 40], dt.bfloat16, tag='ptile', name='cps')
                        ct9 = sb2.tile([9, 4, 128], dt.bfloat16, tag='ct9', name='ct9')
                        for j in range(4):
                            dma(ct9[:, j, :], coefp[j][ri:ri + 9, pc * 128:(pc + 1) * 128])
                        for j in range(4):
                            nc.tensor.transpose(cps[:, j * 10:j * 10 + 9],
                                                ct9[0:9, j, :],
                                                identb[0:9, 0:9])
                        coefT = sb2.tile([128, 40], dt.float32, tag='coefT', name='coefT')
                        for j in range(4):
                            nc.vector.tensor_copy(coefT[:, j * 10:j * 10 + 9], cps[:, j * 10:j * 10 + 9])
                        # scale + sum -> sampT [128, 2304]
                        sampT = sb.tile([128, 9, 256], dt.bfloat16, tag='sampT', name='sampT')
                        for k in range(9):
                            t0 = sb2.tile([128, 256], dt.bfloat16, tag='t0', name='t0')
                            t1 = sb2.tile([128, 256], dt.bfloat16, tag='t1', name='t1')
                            nc.vector.tensor_scalar(t0[:], G[:, k * 4 + 0, :], coefT[:, k:k + 1], None, AT.mult)
                            nc.vector.tensor_scalar(t1[:], G[:, k * 4 + 1, :], coefT[:, 10 + k:11 + k], None, AT.mult)
                            nc.vector.tensor_tensor(t0[:], t0[:], t1[:], AT.add)
                            nc.vector.tensor_scalar(t1[:], G[:, k * 4 + 2, :], coefT[:, 20 + k:21 + k], None, AT.mult)
                            nc.vector.tensor_tensor(t0[:], t0[:], t1[:], AT.add)
                            nc.vector.tensor_scalar(t1[:], G[:, k * 4 + 3, :], coefT[:, 30 + k:31 + k], None, AT.mult)
                            nc.vector.tensor_tensor(sampT[:, k, :], t0[:], t1[:], AT.add)
                        if PART < 3:
                            nc.gpsimd.dma_start(do[0:128, pc * 128:(pc + 1) * 128],
                                                sampT[:, 0, 0:128])
                            continue
                        # transpose to c-major
                        scm = sb.tile([128, 18, 128], dt.bfloat16, tag='scm', name='scm')
                        for grp in range(5):
                            g0 = grp * 4
                            g1 = min(g0 + 4, 18)
                            tps = ps1.tile([128, 512], dt.bfloat16, tag='tps', name='tps')
                            for blk in range(g0, g1):
                                nc.tensor.transpose(tps[:, (blk - g0) * 128:(blk - g0 + 1) * 128],
                                                    sampT[:].rearrange('p a b -> p (a b)')[:, blk * 128:(blk + 1) * 128],
                                                    identb[:, :])
                            nc.vector.tensor_copy(scm[:].rearrange('p a b -> p (a b)')[:, g0 * 128:g1 * 128],
                                                  tps[:, 0:(g1 - g0) * 128])
                        if PART < 4:
                            nc.gpsimd.dma_start(do[0:128, pc * 128:(pc + 1) * 128],
                                                scm[:, 0, :])
                            continue
                        # dcn matmul
                        for mh in range(2):
                            po = ps.tile([128, 128], dt.float32, tag='po', name=f'po{mh}')
                            for blk in range(18):
                                nc.tensor.matmul(po[:],
                                                 wdtt[:, blk * 256 + mh * 128: blk * 256 + (mh + 1) * 128],
                                                 scm[:, blk, :],
                                                 start=(blk == 0), stop=(blk == 17))
                            if PART < 5:
                                ot = sb.tile([128, 128], dt.float32, tag='ot', name='ot')
                                nc.scalar.activation(ot[:], po[:], AF.Copy)
                                dma(do[mh * 128:(mh + 1) * 128, pc * 128:(pc + 1) * 128], ot[:])
                                continue
                            # stats (simple verified ops)
                            mtile = sb.tile([128, 128], dt.float32, tag='mtile', name='mtile')
                            dma(mtile[:], P['maskb'][ti, :, pc * 128:(pc + 1) * 128])
                            msc = sb2.tile([128, 128], dt.float32, tag='msc', name='msc')
                            acc = sb2.tile([128, 1], dt.float32, tag='acc', name='acc')
                            ac2 = sb2.tile([128, 1], dt.float32, tag='ac2', name='ac2')
                            nc.vector.tensor_tensor(msc[:], po[:], mtile[:], AT.mult)
                            nc.vector.tensor_reduce(acc[:], msc[:], axis=AX.X, op=AT.add)
                            nc.vector.tensor_scalar(ac2[:], acc[:], 1.0 / Npos_full, None, AT.mult)
                            sc = stat_col(ti, b, mh, 's1')
                            nc.vector.tensor_tensor(stats[:, sc:sc + 1], stats[:, sc:sc + 1], ac2[:], AT.add)
                            sq = sb.tile([128, 128], dt.float32, tag='sq', name='sq')
                            nc.scalar.activation(sq[:], po[:], AF.Square)
                            nc.vector.tensor_tensor(msc[:], sq[:], mtile[:], AT.mult)
                            nc.vector.tensor_reduce(acc[:], msc[:], axis=AX.X, op=AT.add)
                            nc.vector.tensor_scalar(ac2[:], acc[:], 1.0 / Npos_full, None, AT.mult)
                            sc = stat_col(ti, b, mh, 's2')
                            nc.vector.tensor_tensor(stats[:, sc:sc + 1], stats[:, sc:sc + 1], ac2[:], AT.add)
                            if t['br'] == 'hi':
                                hi_ix = [1, 4].index(ti)
                                otile = sb.tile([128, 128], dt.float32, tag='otile', name='otile')
                                dma(otile[:], P['omegab'][hi_ix, :, pc * 128:(pc + 1) * 128])
                                nc.vector.tensor_tensor(msc[:], po[:], otile[:], AT.mult)
                                nc.vector.tensor_reduce(acc[:], msc[:], axis=AX.X, op=AT.add)
                                sc = stat_col(ti, b, mh, 'w')
                                nc.vector.tensor_tensor(stats[:, sc:sc + 1], stats[:, sc:sc + 1], acc[:], AT.add)
                            ot = sb.tile([128, 128], dt.float32, tag='ot', name='ot')
                            nc.scalar.activation(ot[:], po[:], AF.Copy)
                            dma(do[mh * 128:(mh + 1) * 128, pc * 128:(pc + 1) * 128], ot[:])

            if STAGE < 3:
                raise _StopBuild()
            # ========= PHASE E: allreduce ==========
            cci = dram.tile([128, 64], dt.float32)
            cco = dram.tile([128, 64], dt.float32)
            dma(cci[:], stats[:])
            nc.gpsimd.collective_compute(
                'AllReduce', AT.add, replica_groups=[list(range(NCOR))],
                ins=[cci.opt()], outs=[cco.opt()])
            allr = sb.tile([128, 64], dt.float32)
            dma(allr[:], cco[:])

            if STAGE < 4:
                raise _StopBuild()
            # ========= PHASE F: stats math ==========
            indt = sb.tile([128, 8], dt.float32, tag='indt', name='indt')
            dma(indt[:], P['indt'][:])
            ind8 = sb.tile([8, 128], dt.float32, tag='ind8', name='ind8')
            dma(ind8[:], P['ind8'][:])
            gmt = sb.tile([128, 6], dt.float32)
            dma(gmt[:].rearrange('c (w m) -> c w m', w=3), P['gamma'][:].rearrange('w (m c) -> c w m', m=2, c=128))
            bmt = sb.tile([128, 6], dt.float32)
            dma(bmt[:].rearrange('c (w m) -> c w m', w=3), P['beta'][:].rearrange('w (m c) -> c w m', m=2, c=128))
            # group sums of cols 0..55
            pg = ps.tile([8, 64], dt.float32, tag='psmall', name='pg')
            nc.tensor.matmul(pg[:, 0:56], indt[:], allr[:, 0:56], start=True, stop=True)
            grp = sb.tile([8, 64], dt.float32)
            nc.scalar.activation(grp[:], pg[:], AF.Copy)
            # mu_g = S1g/16 ; var = S2g/16 - mu^2 ; rs = 1/sqrt(var+eps)
            mu = sb.tile([8, 28], dt.float32)
            tss(mu[:], grp[:, 0:28], 1.0 / 16.0, AT.mult)
            var = sb.tile([8, 28], dt.float32)
            tss(var[:], grp[:, 28:56], 1.0 / 16.0, AT.mult)
            mu2 = sb.tile([8, 28], dt.float32)
            nc.vector.tensor_tensor(mu2[:], mu[:], mu[:], AT.mult)
            nc.vector.tensor_tensor(var[:], var[:], mu2[:], AT.subtract)
            tss(var[:], var[:], EPS, AT.add)
            sd = sb.tile([8, 28], dt.float32)
            nc.scalar.activation(sd[:], var[:], AF.Sqrt)
            rs = sb.tile([8, 28], dt.float32)
            nc.vector.reciprocal(rs[:], sd[:])
            # expand to channels
            pex = ps.tile([128, 56], dt.float32, tag='psmall', name='pex')
            nc.tensor.matmul(pex[:, 0:28], ind8[:], rs[:], start=True, stop=True)
            nc.tensor.matmul(pex[:, 28:56], ind8[:], mu[:], start=True, stop=True)
            rse = sb.tile([128, 28], dt.float32)
            nc.scalar.activation(rse[:], pex[:, 0:28], AF.Copy)
            mue = sb.tile([128, 28], dt.float32)
            nc.scalar.activation(mue[:], pex[:, 28:56], AF.Copy)
            # scale/shift per col (ti,b,mh): s = gamma*rs ; t = beta - mu*s
            sc_t = sb.tile([128, 28], dt.float32)
            tc_t = sb.tile([128, 28], dt.float32)
            for ti, t in enumerate(cfg.tinfo):
                for b in range(B):
                    for mh in range(2):
                        col = (ti * 2 + b) * 2 + mh
                        gcol = gmt[:, t['wi'] * 2 + mh: t['wi'] * 2 + mh + 1]
                        bcol = bmt[:, t['wi'] * 2 + mh: t['wi'] * 2 + mh + 1]
                        nc.vector.tensor_scalar(sc_t[:, col:col + 1], rse[:, col:col + 1], gcol, None, AT.mult)
                        nc.vector.tensor_scalar(tc_t[:, col:col + 1], mue[:, col:col + 1], gcol, None, AT.mult)
                        nc.vector.tensor_tensor(tc_t[:, col:col + 1], tc_t[:, col:col + 1], rse[:, col:col + 1], AT.mult)
                        tss(tc_t[:, col:col + 1], tc_t[:, col:col + 1], -1.0, AT.mult)
                        nc.vector.tensor_scalar(tc_t[:, col:col + 1], tc_t[:, col:col + 1], bcol, None, AT.add)
            # per-channel means of GN'd feats: gap = s*S1' + t
            gap = sb.tile([128, 28], dt.float32)
            nc.vector.tensor_tensor(gap[:], sc_t[:], allr[:, 0:28], AT.mult)
            nc.vector.tensor_tensor(gap[:], gap[:], tc_t[:], AT.add)
            # hi resized means: mr = s*Wsum + t   (cols: hi_ix,b,mh -> 8)
            mr = sb.tile([128, 8], dt.float32)
            for hi_ix, ti in enumerate([1, 4]):
                for b in range(B):
                    for mh in range(2):
                        col = (ti * 2 + b) * 2 + mh
                        wcl = 56 + (hi_ix * 2 + b) * 2 + mh
                        mcol = (hi_ix * 2 + b) * 2 + mh
                        nc.vector.tensor_tensor(mr[:, mcol:mcol + 1], sc_t[:, col:col + 1], allr[:, wcl:wcl + 1], AT.mult)
                        nc.vector.tensor_tensor(mr[:, mcol:mcol + 1], mr[:, mcol:mcol + 1], tc_t[:, col:col + 1], AT.add)
            # attn: per (ti,b): hsig(relu(sw . gap_tib + sb))
            swc = sb.tile([128, 2], dt.float32, tag='swc', name='swc')
            dma(swc[:], P['swcol'][:])
            swb = sb.tile([14, 1], dt.float32, tag='swb', name='swb')
            dma(swb[:], P['swb'][:])
            pat = ps.tile([14, 1], dt.float32, tag='psmall', name='pat')
            nc.tensor.matmul(pat[:], gap[:, 0:28:2], swc[:, 0:1], start=True, stop=False)
            nc.tensor.matmul(pat[:], gap[:, 1:28:2], swc[:, 1:2], start=False, stop=True)
            att = sb.tile([14, 1], dt.float32)
            nc.scalar.activation(att[:], pat[:], AF.Copy)
            nc.vector.tensor_tensor(att[:], att[:], swb[:], AT.add)
            tss(att[:], att[:], 0.0, AT.max)
            tss(att[:], att[:], 3.0, AT.add, 1.0 / 6.0, AT.mult)
            tss(att[:], att[:], 0.0, AT.max)
            tss(att[:], att[:], 1.0, AT.min)
            ones14 = sb.tile([14, 128], dt.float32)
            dma(ones14[:], P['ones14'][:])
            dI = sb.tile([14, 14], dt.float32)
            nc.vector.tensor_scalar(dI[:], ident[0:14, 0:14], att[0:14, :], None, AT.mult)
            pA = ps.tile([128, 14], dt.float32, tag='psmall', name='pA')
            nc.tensor.matmul(pA[:], ones14[:], dI[:], start=True, stop=True)
            attx_all = sb.tile([128, 14], dt.float32)
            nc.scalar.activation(attx_all[:], pA[:], AF.Copy)
            attx = {}
            for ti in range(7):
                for b in range(B):
                    attx[(ti, b)] = attx_all[:, ti * 2 + b:ti * 2 + b + 1]
            # dyrelu coefs per (l, b)
            dy1t = sb.tile([128, 128], dt.float32, tag='dy1t', name='dy1t')
            dma(dy1t[:], P['dy1t'][:])
            dy1b = sb.tile([64, 1], dt.float32, tag='dy1b', name='dy1b')
            dma(dy1b[:], P['dy1b'][:])
            dy2t = sb.tile([64, 1024], dt.float32, tag='dy2t', name='dy2t')
            dma(dy2t[:], P['dy2t'][:])
            dy2b = sb.tile([128, 8], dt.float32, tag='dy2b', name='dy2b')
            dma(dy2b[:], P['dy2b'][:])
            AB = {}
            for l in range(3):
                tis = cfg.ltasks[l]
                nb = float(len(tis))
                for b in range(B):
                    go = sb2.tile([128, 2], dt.float32, tag='go', name='go')
                    nc.vector.memset(go[:], 0.0)
                    for ti in tis:
                        t = cfg.tinfo[ti]
                        for mh in range(2):
                            col = (ti * 2 + b) * 2 + mh
                            src = gap[:, col:col + 1]
                            if t['br'] == 'hi':
                                hi_ix = [1, 4].index(ti)
                                src = mr[:, (hi_ix * 2 + b) * 2 + mh:(hi_ix * 2 + b) * 2 + mh + 1]
                            hsc = sb2.tile([128, 1], dt.float32, tag='hsc', name='hsc')
                            nc.vector.tensor_scalar(hsc[:], src, attx[(ti, b)], None, AT.mult)
                            nc.vector.tensor_tensor(go[:, mh:mh + 1], go[:, mh:mh + 1], hsc[:], AT.add)
                    tss(go[:], go[:], 1.0 / nb, AT.mult)
                    ph = ps.tile([64, 1], dt.float32, tag='psmall', name='ph')
                    nc.tensor.matmul(ph[:], dy1t[:, 0:64], go[:, 0:1], start=True, stop=False)
                    nc.tensor.matmul(ph[:], dy1t[:, 64:128], go[:, 1:2], start=False, stop=True)
                    h = sb2.tile([64, 1], dt.float32, tag='h', name='h')
                    nc.scalar.activation(h[:], ph[:], AF.Copy)
                    nc.vector.tensor_tensor(h[:], h[:], dy1b[:], AT.add)
                    tss(h[:], h[:], 0.0, AT.max)
                    pc8 = ps.tile([128, 8], dt.float32, tag='psmall', name='pc8')
                    for mt in range(8):
                        nc.tensor.matmul(pc8[:, mt:mt + 1], dy2t[:, mt * 128:(mt + 1) * 128], h[:], start=True, stop=True)
                    cf8 = sb2.tile([128, 8], dt.float32, tag='cf8', name='cf8')
                    nc.scalar.activation(cf8[:], pc8[:], AF.Copy)
                    nc.vector.tensor_tensor(cf8[:], cf8[:], dy2b[:], AT.add)
                    tss(cf8[:], cf8[:], 3.0, AT.add, 1.0 / 6.0, AT.mult)
                    tss(cf8[:], cf8[:], 0.0, AT.max)
                    tss(cf8[:], cf8[:], 1.0, AT.min)
                    ab = sb.tile([128, 8], dt.float32, tag=f'ab{l}{b}', name=f'ab{l}{b}')
                    tss(ab[:, 0:2], cf8[:, 0:2], -0.5, AT.add, 2.0, AT.mult)
                    tss(ab[:, 0:2], ab[:, 0:2], 1.0, AT.add)
                    tss(ab[:, 2:4], cf8[:, 2:4], -0.5, AT.add)
                    tss(ab[:, 4:6], cf8[:, 4:6], -0.5, AT.add, 2.0, AT.mult)
                    tss(ab[:, 6:8], cf8[:, 6:8], -0.5, AT.add)
                    AB[(l, b)] = ab

            if STAGE < 5:
                raise _StopBuild()
            # ========= PHASE G: GN apply, resize, mean, dyrelu, out ========
            for l in range(3):
                tis = cfg.ltasks[l]
                NpadM = cfg.tinfo[tis[0]]['Npad']
                for b in range(B):
                    macc = {}
                    for mh in range(2):
                        mt = sb.tile([128, NpadM], dt.float32, tag=f'macc{mh}', name=f'macc{mh}')
                        nc.vector.memset(mt[:], 0.0)
                        macc[mh] = mt
                    for ti in tis:
                        t = cfg.tinfo[ti]
                        col0 = (ti * 2 + b) * 2
                        if t['br'] != 'hi':
                            for mh in range(2):
                                ld = sb2.tile([128, NpadM], dt.float32, tag='ld', name='ld')
                                dma(ld[:], dcnout[(ti, b)][mh * 128:(mh + 1) * 128, :])
                                nc.vector.tensor_scalar(ld[:], ld[:], sc_t[:, col0 + mh:col0 + mh + 1],
                                                        tc_t[:, col0 + mh:col0 + mh + 1], AT.mult, AT.add)
                                nc.vector.tensor_scalar(ld[:], ld[:], attx[(ti, b)], None, AT.mult)
                                nc.vector.tensor_tensor(macc[mh][:], macc[mh][:], ld[:], AT.add)
                        else:
                            # GN -> channel-last dram bounce -> static resize gather
                            rz = cfg.resize[l]
                            npch_c = t['Npad'] // 128
                            hcl = dram.tile([t['Npad'], 256], dt.bfloat16, tag=f'hcl{l}{b}')
                            for mh in range(2):
                                ld = sb2.tile([128, t['Npad']], dt.float32, tag='ld')
                                dma(ld[:], dcnout[(ti, b)][mh * 128:(mh + 1) * 128, :])
                                nc.vector.tensor_scalar(ld[:], ld[:], sc_t[:, col0 + mh:col0 + mh + 1],
                                                        tc_t[:, col0 + mh:col0 + mh + 1], AT.mult, AT.add)
                                ldb = sb2.tile([128, t['Npad']], dt.bfloat16, tag='ldb')
                                nc.vector.tensor_copy(ldb[:], ld[:])
                                for pc in range(npch_c):
                                    ptr = ps.tile([128, 128], dt.bfloat16, tag='ptile', name='ptr')
                                    nc.tensor.transpose(ptr[:], ldb[:, pc * 128:(pc + 1) * 128], identb[:])
                                    trs = sb2.tile([128, 128], dt.bfloat16, tag='trs', name='trs')
                                    nc.vector.tensor_copy(trs[:], ptr[:])
                                    dma(hcl[pc * 128:(pc + 1) * 128, mh * 128:(mh + 1) * 128], trs[:])
                            npch_f = rz['Npad'] // 128
                            rzi = sb.tile([128, npch_f * 32], dt.int16, tag=f'rzi{l}', name=f'rzi{l}')
                            dma(rzi[:], P[f'rzidx{l}'][:])
                            rzc = sb.tile([128, npch_f * 4], dt.float32, tag=f'rzc{l}', name=f'rzc{l}')
                            dma(rzc[:], P[f'rzcoef{l}'][:])
                            nh = (npch_f + 1) // 2
                            Gz = sb.tile([128, nh * 4, 256], dt.bfloat16, tag='Gz', name='Gz')
                            Gz2 = sb.tile([128, nh * 4, 256], dt.bfloat16, tag='Gz2', name='Gz2')
                            tot = npch_f * 512
                            off = 0
                            while off < tot:
                                nn = min(512, tot - off)
                                dst = Gz if off < nh * 512 else Gz2
                                dof = off if off < nh * 512 else off - nh * 512
                                nc.gpsimd.dma_gather(
                                    dst[:, dof // 128:(dof + nn) // 128, :], hcl[:],
                                    rzi[:, off // 16:(off + nn) // 16],
                                    num_idxs=nn, num_idxs_reg=nn,
                                    elem_size=256, queue_num=qn[0] % 4)
                                qn[0] += 1
                                off += nn
                            for pc in range(npch_f):
                                t0 = sb2.tile([128, 256], dt.bfloat16, tag='t0', name='t0')
                                t1 = sb2.tile([128, 256], dt.bfloat16, tag='t1', name='t1')
                                Gzc = Gz if pc < nh else Gz2
                                pcl = pc if pc < nh else pc - nh
                                nc.vector.tensor_scalar(t0[:], Gzc[:, pcl * 4 + 0, :], rzc[:, pc * 4:pc * 4 + 1], None, AT.mult)
                                nc.vector.tensor_scalar(t1[:], Gzc[:, pcl * 4 + 1, :], rzc[:, pc * 4 + 1:pc * 4 + 2], None, AT.mult)
                                nc.vector.tensor_tensor(t0[:], t0[:], t1[:], AT.add)
                                nc.vector.tensor_scalar(t1[:], Gzc[:, pcl * 4 + 2, :], rzc[:, pc * 4 + 2:pc * 4 + 3], None, AT.mult)
                                nc.vector.tensor_tensor(t0[:], t0[:], t1[:], AT.add)
                                nc.vector.tensor_scalar(t1[:], Gzc[:, pcl * 4 + 3, :], rzc[:, pc * 4 + 3:pc * 4 + 4], None, AT.mult)
                                nc.vector.tensor_tensor(t0[:], t0[:], t1[:], AT.add)
                                for mh in range(2):
                                    ptz = ps.tile([128, 128], dt.bfloat16, tag='ptile', name='ptz')
                                    nc.tensor.transpose(ptz[:], t0[:, mh * 128:(mh + 1) * 128], identb[:])
                                    rzs = sb2.tile([128, 128], dt.float32, tag='rzs', name='rzs')
                                    nc.vector.tensor_scalar(rzs[:], ptz[:], attx[(ti, b)], None, AT.mult)
                                    nc.vector.tensor_tensor(macc[mh][:, pc * 128:(pc + 1) * 128],
                                                            macc[mh][:, pc * 128:(pc + 1) * 128], rzs[:], AT.add)
                    nbi = 1.0 / len(tis)
                    ab = AB[(l, b)]
                    for mh in range(2):
                        tss(macc[mh][:], macc[mh][:], nbi, AT.mult)
                        o1 = sb.tile([128, NpadM], dt.float32, tag='o1', name='o1')
                        nc.vector.tensor_scalar(o1[:], macc[mh][:], ab[:, 0 + mh:1 + mh],
                                                ab[:, 2 + mh:3 + mh], AT.mult, AT.add)
                        o2 = sb2.tile([128, NpadM], dt.float32, tag='ld', name='o2')
                        nc.vector.tensor_scalar(o2[:], macc[mh][:], ab[:, 4 + mh:5 + mh],
                                                ab[:, 6 + mh:7 + mh], AT.mult, AT.add)
                        nc.vector.tensor_tensor(o1[:], o1[:], o2[:], AT.max)
                        NW = cfg.fine[l]['span'] * HW_L[l][1]
                        dma(OUT[l][b, mh * 128:(mh + 1) * 128, :], o1[:, 0:NW])
    nc.compile()
    return nc


# ===========================================================================
# entry point
# ===========================================================================

def kernel(**inputs):
    import sys
    if '/opt/trn_rl_repo' not in sys.path:
        sys.path.insert(0, '/opt/trn_rl_repo')
    from concourse.bass_utils import run_bass_kernel_spmd
    cfg = CFG
    in_maps = host_prep(inputs, cfg)
    if 'nc' not in _CACHE:
        _CACHE['nc'] = build_nc(cfg)
    nc = _CACHE['nc']
    res = run_bass_kernel_spmd(nc, in_maps, core_ids=list(range(NCOR)))
    outs = []
    for l in range(3):
        H, W = HW_L[l]
        full = np.zeros((B, C, H, W), np.float32)
        f = cfg.fine[l]
        for c in range(NCOR):
            o = res.results[c][f'out{l}'].reshape(B, C, f['span'], W)
            s, e, w0 = f['s'][c], f['e'][c], f['win'][c]
            full[:, :, s:e, :] = o[:, :, s - w0:e - w0, :]
        outs.append(full)
    return tuple(outs)


# revision 41
# speedup vs baseline: 1.0514x; 1.0514x over previous
"""DyHeadBlock Trainium2 kernel: 8-core row-sharded Bass/Tile implementation.

kernel(**inputs) -> (out0, out1, out2) matching reference.reference(**inputs).
Self-contained: hardcodes all shapes/sharding.
"""
import numpy as np

B, C = 2, 256
HW_L = [(80, 80), (40, 40), (20, 20)]
NCOR = 8
EPS = 1e-5

_CACHE = {}


def _shards(H):
    return [c * H // NCOR for c in range(NCOR)], [(c + 1) * H // NCOR for c in range(NCOR)]


class _C:
    pass


def build_cfg():
    cfg = _C()
    cfg.fine, cfg.coarse = [], []
    for l in range(3):
        H, W = HW_L[l]
        s, e = _shards(H)
        span = max(e[c] - s[c] for c in range(NCOR))
        win = [max(0, min(s[c], H - span)) for c in range(NCOR)]
        cfg.fine.append(dict(s=s, e=e, span=span, win=win))
    for l in range(2):
        Hf = HW_L[l][0]
        Hc = HW_L[l + 1][0]
        s, e = _shards(Hc)
        lo, hi = [], []
        for c in range(NCOR):
            f = cfg.fine[l]
            fr = np.arange(f['win'][c], f['win'][c] + f['span'])
            ys = fr * (Hc - 1) / (Hf - 1)
            lo.append(min(int(np.floor(ys.min())), s[c]))
            hi.append(max(min(int(np.floor(ys.max())) + 1, Hc - 1), e[c] - 1))
        span = max(h - l0 + 1 for h, l0 in zip(hi, lo))
        win = [max(0, min(lo[c], Hc - span)) for c in range(NCOR)]
        cfg.coarse.append(dict(s=s, e=e, span=span, win=win))
    # tasks: (level, branch) -> output grid shard + input info
    cfg.tinfo = []
    for l in range(3):
        defs = [(l, 'mid', l, 1, 0)]
        if l > 0:
            defs.append((l, 'low', l - 1, 2, 1))
        if l < 2:
            defs.append((l, 'hi', l + 1, 1, 2))
        for (ll, br, sl, st, wi) in defs:
            if br == 'hi':
                Hg, Wg = HW_L[ll + 1]
                span = cfg.coarse[ll]['span']
            else:
                Hg, Wg = HW_L[ll]
                span = cfg.fine[ll]['span']
            Hin, Win = HW_L[sl]
            N = span * Wg
            Npad = -(-N // 128) * 128
            cfg.tinfo.append(dict(l=ll, br=br, sl=sl, st=st, wi=wi, span=span,
                                  W=Wg, Hg=Hg, Hin=Hin, Win=Win, N=N, Npad=Npad))
    cfg.NT = 7
    cfg.NP = max(t['Npad'] for t in cfg.tinfo)
    # gather slabs (channel-last source rows per task)
    cfg.slab = []
    for ti, t in enumerate(cfg.tinfo):
        Hin = t['Hin']
        lo, hi = [], []
        for c in range(NCOR):
            w0 = (cfg.coarse[t['l']] if t['br'] == 'hi' else cfg.fine[t['l']])['win'][c]
            lo.append(max(0, w0 * t['st'] - 6))
            hi.append(min(Hin - 1, (w0 + t['span'] - 1) * t['st'] + 6))
        span = max(h - l0 + 1 for h, l0 in zip(hi, lo))
        win = [max(0, min(lo[c], Hin - span)) for c in range(NCOR)]
        cfg.slab.append(dict(span=span, win=win))
    cfg.resize = []
    for l in range(2):
        f = cfg.fine[l]
        Hc, Wc = HW_L[l + 1]
        Hf, Wf = HW_L[l]
        N = f['span'] * Wf
        cfg.resize.append(dict(l=l, span=f['span'], Wf=Wf, Hf=Hf, Hc=Hc, Wc=Wc,
                               N=N, Npad=-(-N // 128) * 128))
    # level -> task ids
    cfg.ltasks = [[ti for ti, t in enumerate(cfg.tinfo) if t['l'] == l] for l in range(3)]
    return cfg


CFG = build_cfg()


def _bf16():
    import ml_dtypes
    return ml_dtypes.bfloat16


# ===========================================================================
# host prep
# ===========================================================================

def host_prep(inputs, cfg):
    xs = [np.asarray(inputs['x0'], np.float32), np.asarray(inputs['x1'], np.float32),
          np.asarray(inputs['x2'], np.float32)]
    off_w = np.asarray(inputs['off_w'], np.float32)
    off_b = np.asarray(inputs['off_b'], np.float32)
    shared = {}
    # permute offset conv out-channels: [dy0..8, dx0..8, mask0..8]
    perm = np.concatenate([np.arange(0, 18, 2), np.arange(1, 18, 2), np.arange(18, 27)])
    off_w = off_w[perm]
    off_b = off_b[perm]
    offw_t = np.zeros((9, 2, 128, 32), np.float32)
    for k in range(9):
        w = off_w[:, :, k // 3, k % 3]
        offw_t[k, 0, :, :27] = w[:, :128].T
        offw_t[k, 1, :, :27] = w[:, 128:].T
    shared['offw_t'] = offw_t.reshape(9 * 2 * 128, 32).astype(_bf16())
    for wi, wn in enumerate(['w_mid', 'w_low', 'w_high']):
        w = np.asarray(inputs[wn], np.float32).reshape(C, C, 9)
        tt = np.zeros((18, 128, 256), np.float32)
        for k in range(9):
            for ch in range(2):
                tt[k * 2 + ch] = w[:, ch * 128:(ch + 1) * 128, k].T
        shared[f'wd{wi}'] = tt.reshape(18 * 128, 256).astype(_bf16())
    shared['gamma'] = np.stack([np.asarray(inputs[g], np.float32) for g in
                                ['g_mid', 'g_low', 'g_high']])       # [3, 256]
    shared['beta'] = np.stack([np.asarray(inputs[g], np.float32) for g in
                               ['be_mid', 'be_low', 'be_high']])
    shared['swcol'] = np.asarray(inputs['scale_w'], np.float32).reshape(2, 128).T.copy()  # [128, 2]
    shared['swb'] = np.full((14, 1), float(np.asarray(inputs['scale_b']).reshape(())), np.float32)
    shared['dy1t'] = np.asarray(inputs['dy1_w'], np.float32).reshape(64, 2, 128).transpose(2, 1, 0).reshape(128, 128).copy()
    shared['dy1b'] = np.asarray(inputs['dy1_b'], np.float32).reshape(64, 1)
    shared['dy2t'] = np.asarray(inputs['dy2_w'], np.float32).T.copy()  # [64, 1024]
    shared['dy2b'] = np.asarray(inputs['dy2_b'], np.float32).reshape(8, 128).T.copy()  # [128, 8]
    indt = np.zeros((128, 8), np.float32)
    indt[np.arange(128), np.arange(128) // 16] = 1.0
    shared['indt'] = indt
    ind8 = np.zeros((8, 128), np.float32)
    for g in range(8):
        ind8[g, g * 16:(g + 1) * 16] = 1.0
    shared['ind8'] = ind8
    shared['ident'] = np.eye(128, dtype=np.float32)
    shared['ones14'] = np.ones((14, 128), np.float32)
    msel = np.zeros((128, 8, 128), np.float32)
    for u in range(128):
        for q in range(128):
            for j in range(8):
                if u == 16 * j + (q % 16):
                    msel[u, j, q] = 1.0
    shared['msel'] = msel.reshape(128, 8 * 128)
    shared['off_b'] = np.pad(off_b, (0, 5)).reshape(32, 1)

    in_maps = []
    for c in range(NCOR):
        m = dict(shared)
        # cm slabs: fine windows (+1 row halo each side), zero-padded cols
        for l in range(3):
            H, W = HW_L[l]
            fw = cfg.fine[l]['win'][c]
            rows = cfg.fine[l]['span'] + 2
            slab = np.zeros((B, C, rows, W + 2), np.float32)
            r0 = max(0, fw - 1)
            r1 = min(H, fw - 1 + rows)
            slab[:, :, r0 - (fw - 1):r1 - (fw - 1), 1:W + 1] = xs[l][:, :, r0:r1, :]
            m[f'xcm{l}'] = slab.reshape(B, C, rows * (W + 2)).astype(_bf16())
        # hi-grid cm slabs: rows 2*cw-1 .. 2*cw+2*span_c-1
        for l in range(2):
            H, W = HW_L[l]
            cw = cfg.coarse[l]['win'][c]
            spc = cfg.coarse[l]['span']
            rows = 2 * spc + 1
            slab = np.zeros((B, C, rows, W + 2), np.float32)
            r0 = max(0, 2 * cw - 1)
            r1 = min(H, 2 * cw - 1 + rows)
            slab[:, :, r0 - (2 * cw - 1):r1 - (2 * cw - 1), 1:W + 1] = xs[l][:, :, r0:r1, :]
            m[f'xch{l}'] = slab.reshape(B, C, rows * (W + 2)).astype(_bf16())
        for ti, t in enumerate(cfg.tinfo):
            sl = cfg.slab[ti]
            w0 = sl['win'][c]
            sub = xs[t['sl']][:, :, w0:w0 + sl['span'], :]
            m[f'xcl{ti}'] = np.ascontiguousarray(
                sub.transpose(0, 2, 3, 1).reshape(B, sl['span'] * t['Win'], C)).astype(_bf16())
        ky, kx, prow = _pipe_consts(cfg, c)
        m['ky'] = ky
        m['kx'] = kx
        m['prow'] = prow
        masks, omegas = _stats_masks(cfg, c)
        mb = np.zeros((cfg.NT, cfg.NP), np.float32)
        og = np.zeros((2, cfg.NP), np.float32)
        hi_i = 0
        for ti, t in enumerate(cfg.tinfo):
            mb[ti, :t['Npad']] = masks[ti]
            if t['br'] == 'hi':
                og[hi_i, :t['Npad']] = omegas[ti]
                hi_i += 1
        m['maskb'] = np.broadcast_to(mb[:, None, :], (cfg.NT, 128, cfg.NP)).copy()
        m['omegab'] = np.broadcast_to(og[:, None, :], (2, 128, cfg.NP)).copy()
        for l in range(2):
            idx, coef = _resize_consts(cfg, c, l)
            rz = cfg.resize[l]
            npch = rz['Npad'] // 128
            wr = np.zeros((128, npch * 32), np.int16)
            ii = np.arange(npch * 4 * 128)
            pch, blk, pl = ii // 512, (ii // 128) % 4, ii % 128
            wr[ii % 16, ii // 16] = idx[blk, pch * 128 + pl]
            wr[16:] = np.tile(wr[:16], (7, 1))
            m[f'rzidx{l}'] = wr
            cf = np.zeros((128, npch * 4), np.float32)
            for b4 in range(4):
                cf[:, b4::4] = coef[b4].reshape(npch, 128).T
            m[f'rzcoef{l}'] = cf
        in_maps.append(m)
    return in_maps


def _pipe_consts(cfg, c):
    NP = cfg.NP
    ky = np.full((128, NP), 2.25, np.float32)
    kx = np.full((128, NP), 2.25, np.float32)
    prow = np.zeros((128, 4), np.float32)
    for ri in range(126):
        ti, b, k = ri // 18, (ri // 9) % 2, ri % 9
        t = cfg.tinfo[ti]
        kh, kw = k // 3, k % 3
        w0 = (cfg.coarse[t['l']] if t['br'] == 'hi' else cfg.fine[t['l']])['win'][c]
        rows = (np.arange(t['span'])[:, None] + w0) * t['st'] - 1 + kh
        cols = np.arange(t['W'])[None, :] * t['st'] - 1 + kw
        ky[ri, :t['N']] = np.broadcast_to(rows, (t['span'], t['W'])).reshape(-1)
        kx[ri, :t['N']] = np.broadcast_to(cols, (t['span'], t['W'])).reshape(-1)
        ky[ri, t['N']:] = cfg.slab[ti]['win'][c] + 2.25   # pad -> inside slab
        prow[ri] = [t['Hin'] - 1, t['Win'] - 1, t['Win'],
                    -cfg.slab[ti]['win'][c] * t['Win']]
    prow[126:] = [1, 1, 1, 0]
    return ky, kx, prow


def _stats_masks(cfg, c):
    masks, omegas = [], []
    for ti, t in enumerate(cfg.tinfo):
        sh = cfg.coarse[t['l']] if t['br'] == 'hi' else cfg.fine[t['l']]
        w0 = sh['win'][c]
        rows = np.arange(t['span']) + w0
        ownrow = (rows >= sh['s'][c]) & (rows < sh['e'][c])
        mk = np.zeros(t['Npad'], np.float32)
        mk[:t['N']] = np.repeat(ownrow.astype(np.float32), t['W'])
        masks.append(mk)
        if t['br'] == 'hi':
            l = t['l']
            Hc, Wc = HW_L[l + 1]
            Hf, Wf = HW_L[l]
            ys = np.arange(Hf) * (Hc - 1.0) / (Hf - 1.0)
            y0 = np.floor(ys).astype(np.int64)
            y1 = np.minimum(y0 + 1, Hc - 1)
            wy = ys - y0
            xg = np.arange(Wf) * (Wc - 1.0) / (Wf - 1.0)
            x0 = np.floor(xg).astype(np.int64)
            x1 = np.minimum(x0 + 1, Wc - 1)
            wx = xg - x0
            wrow = np.zeros(Hc)
            wcol = np.zeros(Wc)
            np.add.at(wrow, y0, 1 - wy)
            np.add.at(wrow, y1, wy)
            np.add.at(wcol, x0, 1 - wx)
            np.add.at(wcol, x1, wx)
            om_full = np.outer(wrow, wcol) / (Hf * Wf)
            o = np.zeros(t['Npad'], np.float32)
            o[:t['N']] = (om_full[rows] * ownrow[:, None]).reshape(-1)
            omegas.append(o)
        else:
            omegas.append(None)
    return masks, omegas


def _resize_consts(cfg, c, l):
    rz = cfg.resize[l]
    Hc, Wc, Hf, Wf = rz['Hc'], rz['Wc'], rz['Hf'], rz['Wf']
    w0f = cfg.fine[l]['win'][c]
    w0c = cfg.coarse[l]['win'][c]
    spc = cfg.coarse[l]['span']
    rows = np.arange(rz['span']) + w0f
    ys = rows * (Hc - 1.0) / (Hf - 1.0)
    y0 = np.floor(ys).astype(np.int64)
    y1 = np.minimum(y0 + 1, Hc - 1)
    wy = (ys - y0).astype(np.float32)
    xg = np.arange(Wf) * (Wc - 1.0) / (Wf - 1.0)
    x0 = np.floor(xg).astype(np.int64)
    x1 = np.minimum(x0 + 1, Wc - 1)
    wx = (xg - x0).astype(np.float32)
    N, Npad = rz['N'], rz['Npad']
    idx = np.zeros((4, Npad), np.int32)
    coef = np.zeros((4, Npad), np.float32)
    Y0 = np.repeat(y0, Wf); Y1 = np.repeat(y1, Wf); WY = np.repeat(wy, Wf)
    X0 = np.tile(x0, rz['span']); X1 = np.tile(x1, rz['span']); WX = np.tile(wx, rz['span'])

    def rel(y):
        return np.clip(y, w0c, w0c + spc - 1) - w0c

    idx[0, :N] = rel(Y0) * Wc + X0; coef[0, :N] = (1 - WY) * (1 - WX)
    idx[1, :N] = rel(Y0) * Wc + X1; coef[1, :N] = (1 - WY) * WX
    idx[2, :N] = rel(Y1) * Wc + X0; coef[2, :N] = WY * (1 - WX)
    idx[3, :N] = rel(Y1) * Wc + X1; coef[3, :N] = WY * WX
    return idx, coef


# ===========================================================================
# device program
# ===========================================================================

def build_nc(cfg):
    import os
    STAGE = int(os.environ.get('KSTAGE', '9'))
    PART = int(os.environ.get('KPART', '9'))
    KTASK = int(os.environ.get('KTASK', '-1'))
    import concourse.bacc as bacc
    import concourse.mybir as mybir
    import concourse.tile as tile
    from contextlib import ExitStack
    dt = mybir.dt
    AT = mybir.AluOpType
    AF = mybir.ActivationFunctionType
    AX = mybir.AxisListType

    class _StopBuild(Exception):
        pass
    nc = bacc.Bacc('TRN2', target_bir_lowering=False, debug=False, num_devices=NCOR, num_swdge_queues=4)
    NP = cfg.NP

    P = {}

    def par(nm, shp, d):
        P[nm] = nc.declare_dram_parameter(nm, shp, d, isOutput=False)

    par('offw_t', [9 * 2 * 128, 32], dt.bfloat16)
    for wi in range(3):
        par(f'wd{wi}', [18 * 128, 256], dt.bfloat16)
    par('gamma', [3, 256], dt.float32)
    par('beta', [3, 256], dt.float32)
    par('swcol', [128, 2], dt.float32)
    par('swb', [14, 1], dt.float32)
    par('dy1t', [128, 128], dt.float32)
    par('dy1b', [64, 1], dt.float32)
    par('dy2t', [64, 1024], dt.float32)
    par('dy2b', [128, 8], dt.float32)
    par('indt', [128, 8], dt.float32)
    par('ind8', [8, 128], dt.float32)
    par('ident', [128, 128], dt.float32)
    par('ones14', [14, 128], dt.float32)
    par('msel', [128, 8 * 128], dt.float32)
    par('off_b', [32, 1], dt.float32)
    par('ky', [128, NP], dt.float32)
    par('kx', [128, NP], dt.float32)
    par('prow', [128, 4], dt.float32)
    par('maskb', [cfg.NT, 128, NP], dt.float32)
    par('omegab', [2, 128, NP], dt.float32)
    for l in range(3):
        rows = cfg.fine[l]['span'] + 2
        par(f'xcm{l}', [B, C, rows * (HW_L[l][1] + 2)], dt.bfloat16)
    for l in range(2):
        rows = 2 * cfg.coarse[l]['span'] + 1
        par(f'xch{l}', [B, C, rows * (HW_L[l][1] + 2)], dt.bfloat16)
    for ti, t in enumerate(cfg.tinfo):
        par(f'xcl{ti}', [B, cfg.slab[ti]['span'] * t['Win'], C], dt.bfloat16)
    for l in range(2):
        npch = cfg.resize[l]['Npad'] // 128
        par(f'rzidx{l}', [128, npch * 32], dt.int16)
        par(f'rzcoef{l}', [128, npch * 4], dt.float32)
    OUT = [nc.declare_dram_parameter(f'out{l}', [B, C, cfg.fine[l]['span'] * HW_L[l][1]],
                                     dt.float32, isOutput=True) for l in range(3)]

    import contextlib
    with tile.TileContext(nc) as tc:
        with contextlib.suppress(_StopBuild), ExitStack() as ctx:
            sb = ctx.enter_context(tc.tile_pool(name='sb', bufs=1))
            sb2 = ctx.enter_context(tc.tile_pool(name='sb2', bufs=2))
            ps = ctx.enter_context(tc.tile_pool(name='ps', bufs=2, space='PSUM'))
            ps1 = ctx.enter_context(tc.tile_pool(name='ps1', bufs=1, space='PSUM'))
            dram = ctx.enter_context(tc.tile_pool(name='dram', bufs=1, space='DRAM'))

            def dma(dst, src):
                nc.sync.dma_start(dst, src)

            def tss(out, in0, s1, op0, s2=None, op1=None):
                if op1 is None:
                    nc.vector.tensor_scalar(out, in0, s1, None, op0)
                else:
                    nc.vector.tensor_scalar(out, in0, s1, s2, op0, op1)

            # ---- constants
            ident = sb.tile([128, 128], dt.float32)
            dma(ident[:], P['ident'][:])
            identb = sb.tile([128, 128], dt.bfloat16)
            nc.vector.tensor_copy(identb[:], ident[:])
            offw = sb.tile([128, 18 * 32], dt.bfloat16)
            dma(offw[:].rearrange('p (a c) -> p a c', a=18), P['offw_t'][:].rearrange('(a p) c -> p a c', p=128))
            wdt_cur = {'wi': None, 't': None}

            def get_wdt(wi):
                wt_ = sb.tile([128, 18 * 256], dt.bfloat16, tag='wdtcur', name='wdtcur')
                dma(wt_[:].rearrange('p (a c) -> p a c', a=18), P[f'wd{wi}'][:].rearrange('(a p) c -> p a c', p=128))
                return wt_
            ky = sb.tile([128, NP], dt.float32, tag='ky', name='ky')
            dma(ky[:], P['ky'][:])
            kx = sb.tile([128, NP], dt.float32, tag='kx', name='kx')
            dma(kx[:], P['kx'][:])
            prow = sb.tile([128, 4], dt.float32, tag='prow', name='prow')
            dma(prow[:], P['prow'][:])
            offb = sb.tile([32, 1], dt.float32, tag='offb', name='offb')
            dma(offb[:], P['off_b'][:])

            # ========= PHASE A: offset convs ==========
            om_f, om_h = {}, {}

            def offconv(dst, slabpar, b, span, Wout, Win, stride):
                Wp = Win + 2
                srows = (span - 1) * stride + 3
                blk = srows * Wp + 2
                slab = sb.tile([128, 2 * blk], dt.bfloat16, tag='cslab', name='cslab')
                for ct in range(2):
                    dma(slab[:, ct * blk:ct * blk + srows * Wp],
                        slabpar[b, ct * 128:(ct + 1) * 128, :])
                r = 0
                while r < span:
                    rr = min(6 if stride == 1 else 4, span - r)
                    Nch = rr * Wout
                    pt = ps1.tile([32, 512], dt.float32, tag='ompsum', name='ompsum')
                    cnt = 0
                    for k in range(9):
                        kh, kw = k // 3, k % 3
                        for ct in range(2):
                            base = ct * blk + (r * stride + kh) * Wp + kw
                            nsr = (rr - 1) * stride + 1
                            rhs = slab[:, base:base + nsr * Wp]
                            rhs = rhs.rearrange('p (r w) -> p r w', r=nsr, w=Wp)
                            if stride > 1:
                                rhs = rhs[:, ::stride, 0:(Wout - 1) * stride + 1:stride]
                            else:
                                rhs = rhs[:, :, 0:Wout]
                            nc.tensor.matmul(pt[0:27, 0:Nch],
                                             offw[:, (k * 2 + ct) * 32:(k * 2 + ct) * 32 + 27],
                                             rhs, start=(cnt == 0), stop=(cnt == 17))
                            cnt += 1
                    nc.scalar.activation(dst[0:27, r * Wout:r * Wout + Nch], pt[0:27, 0:Nch], AF.Copy)
                    r += rr

            for l in range(3):
                W = HW_L[l][1]
                for b in range(B):
                    of = sb.tile([32, cfg.fine[l]['span'] * W], dt.float32, tag=f'omf{l}{b}')
                    offconv(of, P[f'xcm{l}'], b, cfg.fine[l]['span'], W, W, 1)
                    nc.vector.tensor_scalar(of[0:27, :], of[0:27, :], offb[0:27, :], None, AT.add)
                    om_f[(l, b)] = of
                    if l < 2:
                        Wc = HW_L[l + 1][1]
                        oh = sb.tile([32, cfg.coarse[l]['span'] * Wc], dt.float32, tag=f'omh{l}{b}')
                        offconv(oh, P[f'xch{l}'], b, cfg.coarse[l]['span'], Wc, W, 2)
                        nc.vector.tensor_scalar(oh[0:27, :], oh[0:27, :], offb[0:27, :], None, AT.add)
                        om_h[(l, b)] = oh

            # ========= PHASE B: coef/idx pipeline [126, NP] ==========
            dy = sb.tile([128, NP], dt.float32)
            dx = sb.tile([128, NP], dt.float32)
            msk = sb.tile([128, NP], dt.float32)
            for t3 in (dy, dx, msk):
                nc.vector.memset(t3[:], 0.0)
            for ti, t in enumerate(cfg.tinfo):
                for b in range(B):
                    ri = (ti * 2 + b) * 9
                    src = om_h[(t['l'], b)] if t['br'] == 'hi' else om_f[(t['l'], b)]
                    dma(dy[ri:ri + 9, 0:t['N']], src[0:9, 0:t['N']])
                    dma(dx[ri:ri + 9, 0:t['N']], src[9:18, 0:t['N']])
                    dma(msk[ri:ri + 9, 0:t['N']], src[18:27, 0:t['N']])

            nc.scalar.activation(msk[:], msk[:], AF.Sigmoid)
            Hm1 = prow[:, 0:1]
            Wm1 = prow[:, 1:2]
            Wmul = prow[:, 2:3]
            shift = prow[:, 3:4]
            py = sb.tile([128, NP], dt.float32)
            px = sb.tile([128, NP], dt.float32)
            nc.vector.tensor_tensor(py[:], ky[:], dy[:], AT.add)
            nc.vector.tensor_tensor(px[:], kx[:], dx[:], AT.add)
            tmp = sb.tile([128, NP], dt.float32)
            i16 = sb.tile([128, NP], dt.int16)
            y0f = sb.tile([128, NP], dt.float32)
            x0f = sb.tile([128, NP], dt.float32)
            tss(tmp[:], py[:], 15.5, AT.add)
            nc.vector.tensor_copy(i16[:], tmp[:])
            nc.vector.tensor_copy(y0f[:], i16[:])
            tss(y0f[:], y0f[:], -16.0, AT.add)
            tss(tmp[:], px[:], 15.5, AT.add)
            nc.vector.tensor_copy(i16[:], tmp[:])
            nc.vector.tensor_copy(x0f[:], i16[:])
            tss(x0f[:], x0f[:], -16.0, AT.add)
            ly = py
            lx = px
            nc.vector.tensor_tensor(ly[:], py[:], y0f[:], AT.subtract)
            nc.vector.tensor_tensor(lx[:], px[:], x0f[:], AT.subtract)

            wgt = {}
            for (nmw, base, lfr, mlim, foldmask) in [('y', y0f, ly, Hm1, True),
                                                     ('x', x0f, lx, Wm1, False)]:
                for j in range(2):
                    v = sb.tile([128, NP], dt.float32, tag='pv', name='pv')
                    tss(v[:], base[:], float(j), AT.add)
                    g = sb.tile([128, NP], dt.float32, tag='psmall', name='pg')
                    tss(g[:], v[:], 0.0, AT.is_ge)
                    tss(v[:], v[:], mlim, AT.is_le)
                    nc.vector.tensor_tensor(g[:], g[:], v[:], AT.mult)
                    wtag = {('y', 0): 'dy', ('y', 1): 'dx', ('x', 0): 'wx0', ('x', 1): 'wx1'}[(nmw, j)]
                    w = sb.tile([128, NP], dt.float32, tag=wtag, name=wtag + 'w')
                    if j == 0:
                        tss(w[:], lfr[:], -1.0, AT.mult, 1.0, AT.add)
                    else:
                        nc.vector.tensor_copy(w[:], lfr[:])
                    nc.vector.tensor_tensor(w[:], w[:], g[:], AT.mult)
                    if foldmask:
                        nc.vector.tensor_tensor(w[:], w[:], msk[:], AT.mult)
                    wgt[(nmw, j)] = w
            ycl = {}
            xcl_ = {}
            for (nmw, base, mlim, store) in [('y', y0f, Hm1, ycl), ('x', x0f, Wm1, xcl_)]:
                for j in range(2):
                    vtag = {('y', 0): 'ky', ('y', 1): 'kx', ('x', 0): 'msk', ('x', 1): 'cx1'}[(nmw, j)]
                    v = sb.tile([128, NP], dt.float32, tag=vtag, name=vtag + 'c')
                    tss(v[:], base[:], float(j), AT.add)
                    tss(v[:], v[:], 0.0, AT.max)
                    tss(v[:], v[:], mlim, AT.min)
                    store[j] = v
            coefp = {}
            idxp = {}
            for jy in range(2):
                for jx in range(2):
                    j = jy * 2 + jx
                    cpl = sb.tile([128, NP], dt.bfloat16, tag=f'coef{j}', name=f'coef{j}')
                    nc.vector.tensor_tensor(tmp[:], wgt[('y', jy)][:], wgt[('x', jx)][:], AT.mult)
                    nc.vector.tensor_copy(cpl[:], tmp[:])
                    coefp[j] = cpl
                    ipl = sb.tile([128, NP], dt.float32, tag=f'idx{j}', name=f'idx{j}')
                    nc.vector.tensor_scalar(ipl[:], ycl[jy][:], Wmul, None, AT.mult)
                    nc.vector.tensor_tensor(ipl[:], ipl[:], xcl_[jx][:], AT.add)
                    nc.vector.tensor_scalar(ipl[:], ipl[:], shift, None, AT.add)
                    idxp[j] = ipl

            # ========= PHASE C: wrapped idx tiles (PE transposes + select) =
            wrapped = {}
            for ti, t in enumerate(cfg.tinfo):
                npch = t['Npad'] // 128
                ncols = npch * 36
                L = npch * 288
                for b in range(B):
                    ri = (ti * 2 + b) * 9
                    Tt = sb.tile([128, 252], dt.float32, tag='Tidx', name='Tidx')
                    for pch in range(npch):
                        c9i = sb2.tile([9, 4, 128], dt.float32, tag='c9i', name='c9i')
                        for j in range(4):
                            dma(c9i[:, j, :], idxp[j][ri:ri + 9, pch * 128:(pch + 1) * 128])
                        tw = ps.tile([128, 36], dt.float32, tag='po', name='tw')
                        for j in range(4):
                            nc.tensor.transpose(tw[:, j * 9:(j + 1) * 9], c9i[0:9, j, :], ident[0:9, 0:9])
                        # reorder (j,k) -> blk=k*4+j while copying
                        nc.vector.tensor_copy(
                            Tt[:, pch * 36:(pch + 1) * 36].rearrange('p (k j) -> p k j', k=9, j=4),
                            tw[:].rearrange('p (j k) -> p k j', j=4, k=9))
                    wt = sb.tile([128, L], dt.int16, tag=f'wrap{ti}{b}', name=f'wrap{ti}{b}')
                    for j in range(8):
                        msl = sb2.tile([128, 128], dt.float32, tag='msl', name='msl')
                        dma(msl[:], P['msel'][:, j * 128:(j + 1) * 128])
                        wps = ps.tile([128, 252], dt.float32, tag='po', name='wps')
                        nc.tensor.matmul(wps[:, 0:ncols], msl[:],
                                         Tt[:, 0:ncols], start=True, stop=True)
                        nc.vector.tensor_copy(
                            wt[:].rearrange('p (s j) -> p s j', j=8)[:, :, j],
                            wps[:, 0:ncols])
                    wrapped[(ti, b)] = wt

            # ========= PHASE D: gather + dcn ==========
            if STAGE < 2:
                raise _StopBuild()
            dcnout = {}
            stats = sb.tile([128, 64], dt.float32)
            nc.vector.memset(stats[:], 0.0)
            scol = {}

            def stat_col(ti, b, mh, kind):
                base = {'s1': 0, 's2': 28, 'w': 56}[kind]
                if kind == 'w':
                    hi_ix = [1, 4].index(ti)
                    return base + (hi_ix * 2 + b) * 2 + mh
                return base + (ti * 2 + b) * 2 + mh

            qn = [0]
            for ti, t in enumerate(cfg.tinfo):
                if KTASK >= 0 and ti != KTASK:
                    continue
                npch = t['Npad'] // 128
                Npos_full = t['Hg'] * t['W']
                wdtt = get_wdt(t['wi'])
                for b in range(B):
                    ri = (ti * 2 + b) * 9
                    do = dram.tile([256, t['Npad']], dt.float32, tag=f'do{ti}{b}')
                    dcnout[(ti, b)] = do
                    for pc in range(npch):
                        G = sb2.tile([128, 36, 256], dt.bfloat16, tag='G', name='G')
                        for gch in range(9):
                            nc.gpsimd.dma_gather(
                                G[:, gch * 4:(gch + 1) * 4, :], P[f'xcl{ti}'][b],
                                wrapped[(ti, b)][:, pc * 288 + gch * 32: pc * 288 + (gch + 1) * 32],
                                num_idxs=512, num_idxs_reg=512, elem_size=256,
                                queue_num=qn[0] % 4)
                            qn[0] += 1
                        if PART < 2:
                            nc.gpsimd.dma_start(do[0:128, pc * 128:(pc + 1) * 128],
                                                G[:, 0, 0:128])
                            continue
                        # coef transposes -> coefT [128, 36] bf16
                        cps = ps.tile([128, 40], dt.bfloat16, tag='ptile', name='cps')
                        ct9 = sb2.tile([9, 4, 128], dt.bfloat16, tag='c9i', name='ct9')
                        for j in range(4):
                            dma(ct9[:, j, :], coefp[j][ri:ri + 9, pc * 128:(pc + 1) * 128])
                        for j in range(4):
                            nc.tensor.transpose(cps[:, j * 10:j * 10 + 9],
                                                ct9[0:9, j, :],
                                                identb[0:9, 0:9])
                        coefT = sb2.tile([128, 40], dt.float32, tag='coefT', name='coefT')
                        for j in range(4):
                            nc.vector.tensor_copy(coefT[:, j * 10:j * 10 + 9], cps[:, j * 10:j * 10 + 9])
                        # scale + sum -> sampT [128, 2304]
                        sampT = sb.tile([128, 9, 256], dt.bfloat16, tag='sampT', name='sampT')
                        for k in range(9):
                            t0 = sb2.tile([128, 256], dt.bfloat16, tag='t0', name='t0')
                            t1 = sb2.tile([128, 256], dt.bfloat16, tag='t1', name='t1')
                            nc.vector.tensor_scalar(t0[:], G[:, k * 4 + 0, :], coefT[:, k:k + 1], None, AT.mult)
                            nc.vector.tensor_scalar(t1[:], G[:, k * 4 + 1, :], coefT[:, 10 + k:11 + k], None, AT.mult)
                            nc.vector.tensor_tensor(t0[:], t0[:], t1[:], AT.add)
                            nc.vector.tensor_scalar(t1[:], G[:, k * 4 + 2, :], coefT[:, 20 + k:21 + k], None, AT.mult)
                            nc.vector.tensor_tensor(t0[:], t0[:], t1[:], AT.add)
                            nc.vector.tensor_scalar(t1[:], G[:, k * 4 + 3, :], coefT[:, 30 + k:31 + k], None, AT.mult)
                            nc.vector.tensor_tensor(sampT[:, k, :], t0[:], t1[:], AT.add)
                        if PART < 3:
                            nc.gpsimd.dma_start(do[0:128, pc * 128:(pc + 1) * 128],
                                                sampT[:, 0, 0:128])
                            continue
                        # transpose to c-major
                        scm = sb.tile([128, 18, 128], dt.bfloat16, tag='scm', name='scm')
                        for grp in range(5):
                            g0 = grp * 4
                            g1 = min(g0 + 4, 18)
                            tps = ps1.tile([128, 512], dt.bfloat16, tag='tps', name='tps')
                            for blk in range(g0, g1):
                                nc.tensor.transpose(tps[:, (blk - g0) * 128:(blk - g0 + 1) * 128],
                                                    sampT[:].rearrange('p a b -> p (a b)')[:, blk * 128:(blk + 1) * 128],
                                                    identb[:, :])
                            nc.vector.tensor_copy(scm[:].rearrange('p a b -> p (a b)')[:, g0 * 128:g1 * 128],
                                                  tps[:, 0:(g1 - g0) * 128])
                        if PART < 4:
                            nc.gpsimd.dma_start(do[0:128, pc * 128:(pc + 1) * 128],
                                                scm[:, 0, :])
                            continue
                        # dcn matmul
                        for mh in range(2):
                            po = ps.tile([128, 128], dt.float32, tag='po', name=f'po{mh}')
                            for blk in range(18):
                                nc.tensor.matmul(po[:],
                                                 wdtt[:, blk * 256 + mh * 128: blk * 256 + (mh + 1) * 128],
                                                 scm[:, blk, :],
                                                 start=(blk == 0), stop=(blk == 17))
                            if PART < 5:
                                ot = sb.tile([128, 128], dt.float32, tag='ot', name='ot')
                                nc.scalar.activation(ot[:], po[:], AF.Copy)
                                dma(do[mh * 128:(mh + 1) * 128, pc * 128:(pc + 1) * 128], ot[:])
                                continue
                            # stats (simple verified ops)
                            mtile = sb.tile([128, 128], dt.float32, tag='mtile', name='mtile')
                            dma(mtile[:], P['maskb'][ti, :, pc * 128:(pc + 1) * 128])
                            msc = sb2.tile([128, 128], dt.float32, tag='msc', name='msc')
                            acc = sb2.tile([128, 1], dt.float32, tag='acc', name='acc')
                            ac2 = sb2.tile([128, 1], dt.float32, tag='ac2', name='ac2')
                            nc.vector.tensor_tensor(msc[:], po[:], mtile[:], AT.mult)
                            nc.vector.tensor_reduce(acc[:], msc[:], axis=AX.X, op=AT.add)
                            nc.vector.tensor_scalar(ac2[:], acc[:], 1.0 / Npos_full, None, AT.mult)
                            sc = stat_col(ti, b, mh, 's1')
                            nc.vector.tensor_tensor(stats[:, sc:sc + 1], stats[:, sc:sc + 1], ac2[:], AT.add)
                            sq = sb.tile([128, 128], dt.float32, tag='sq', name='sq')
                            nc.scalar.activation(sq[:], po[:], AF.Square)
                            nc.vector.tensor_tensor(msc[:], sq[:], mtile[:], AT.mult)
                            nc.vector.tensor_reduce(acc[:], msc[:], axis=AX.X, op=AT.add)
                            nc.vector.tensor_scalar(ac2[:], acc[:], 1.0 / Npos_full, None, AT.mult)
                            sc = stat_col(ti, b, mh, 's2')
                            nc.vector.tensor_tensor(stats[:, sc:sc + 1], stats[:, sc:sc + 1], ac2[:], AT.add)
                            if t['br'] == 'hi':
                                hi_ix = [1, 4].index(ti)
                                otile = sb.tile([128, 128], dt.float32, tag='otile', name='otile')
                                dma(otile[:], P['omegab'][hi_ix, :, pc * 128:(pc + 1) * 128])
                                nc.vector.tensor_tensor(msc[:], po[:], otile[:], AT.mult)
                                nc.vector.tensor_reduce(acc[:], msc[:], axis=AX.X, op=AT.add)
                                sc = stat_col(ti, b, mh, 'w')
                                nc.vector.tensor_tensor(stats[:, sc:sc + 1], stats[:, sc:sc + 1], acc[:], AT.add)
                            ot = sb.tile([128, 128], dt.float32, tag='ot', name='ot')
                            nc.scalar.activation(ot[:], po[:], AF.Copy)
                            dma(do[mh * 128:(mh + 1) * 128, pc * 128:(pc + 1) * 128], ot[:])

            if STAGE < 3:
                raise _StopBuild()
            # ========= PHASE E: allreduce ==========
            cci = dram.tile([128, 64], dt.float32)
            cco = dram.tile([128, 64], dt.float32)
            dma(cci[:], stats[:])
            nc.gpsimd.collective_compute(
                'AllReduce', AT.add, replica_groups=[list(range(NCOR))],
                ins=[cci.opt()], outs=[cco.opt()])
            allr = sb.tile([128, 64], dt.float32)
            dma(allr[:], cco[:])

            if STAGE < 4:
                raise _StopBuild()
            # ========= PHASE F: stats math ==========
            indt = sb.tile([128, 8], dt.float32, tag='indt', name='indt')
            dma(indt[:], P['indt'][:])
            ind8 = sb.tile([8, 128], dt.float32, tag='ind8', name='ind8')
            dma(ind8[:], P['ind8'][:])
            gmt = sb.tile([128, 6], dt.float32)
            dma(gmt[:].rearrange('c (w m) -> c w m', w=3), P['gamma'][:].rearrange('w (m c) -> c w m', m=2, c=128))
            bmt = sb.tile([128, 6], dt.float32)
            dma(bmt[:].rearrange('c (w m) -> c w m', w=3), P['beta'][:].rearrange('w (m c) -> c w m', m=2, c=128))
            # group sums of cols 0..55
            pg = ps.tile([8, 64], dt.float32, tag='psmall', name='pg')
            nc.tensor.matmul(pg[:, 0:56], indt[:], allr[:, 0:56], start=True, stop=True)
            grp = sb.tile([8, 64], dt.float32)
            nc.scalar.activation(grp[:], pg[:], AF.Copy)
            # mu_g = S1g/16 ; var = S2g/16 - mu^2 ; rs = 1/sqrt(var+eps)
            mu = sb.tile([8, 28], dt.float32)
            tss(mu[:], grp[:, 0:28], 1.0 / 16.0, AT.mult)
            var = sb.tile([8, 28], dt.float32)
            tss(var[:], grp[:, 28:56], 1.0 / 16.0, AT.mult)
            mu2 = sb.tile([8, 28], dt.float32)
            nc.vector.tensor_tensor(mu2[:], mu[:], mu[:], AT.mult)
            nc.vector.tensor_tensor(var[:], var[:], mu2[:], AT.subtract)
            tss(var[:], var[:], EPS, AT.add)
            sd = sb.tile([8, 28], dt.float32)
            nc.scalar.activation(sd[:], var[:], AF.Sqrt)
            rs = sb.tile([8, 28], dt.float32)
            nc.vector.reciprocal(rs[:], sd[:])
            # expand to channels
            pex = ps.tile([128, 56], dt.float32, tag='psmall', name='pex')
            nc.tensor.matmul(pex[:, 0:28], ind8[:], rs[:], start=True, stop=True)
            nc.tensor.matmul(pex[:, 28:56], ind8[:], mu[:], start=True, stop=True)
            rse = sb.tile([128, 28], dt.float32)
            nc.scalar.activation(rse[:], pex[:, 0:28], AF.Copy)
            mue = sb.tile([128, 28], dt.float32)
            nc.scalar.activation(mue[:], pex[:, 28:56], AF.Copy)
            # scale/shift per col (ti,b,mh): s = gamma*rs ; t = beta - mu*s
            sc_t = sb.tile([128, 28], dt.float32)
            tc_t = sb.tile([128, 28], dt.float32)
            for ti, t in enumerate(cfg.tinfo):
                for b in range(B):
                    for mh in range(2):
                        col = (ti * 2 + b) * 2 + mh
                        gcol = gmt[:, t['wi'] * 2 + mh: t['wi'] * 2 + mh + 1]
                        bcol = bmt[:, t['wi'] * 2 + mh: t['wi'] * 2 + mh + 1]
                        nc.vector.tensor_scalar(sc_t[:, col:col + 1], rse[:, col:col + 1], gcol, None, AT.mult)
                        nc.vector.tensor_scalar(tc_t[:, col:col + 1], mue[:, col:col + 1], gcol, None, AT.mult)
                        nc.vector.tensor_tensor(tc_t[:, col:col + 1], tc_t[:, col:col + 1], rse[:, col:col + 1], AT.mult)
                        tss(tc_t[:, col:col + 1], tc_t[:, col:col + 1], -1.0, AT.mult)
                        nc.vector.tensor_scalar(tc_t[:, col:col + 1], tc_t[:, col:col + 1], bcol, None, AT.add)
            # per-channel means of GN'd feats: gap = s*S1' + t
            gap = sb.tile([128, 28], dt.float32)
            nc.vector.tensor_tensor(gap[:], sc_t[:], allr[:, 0:28], AT.mult)
            nc.vector.tensor_tensor(gap[:], gap[:], tc_t[:], AT.add)
            # hi resized means: mr = s*Wsum + t   (cols: hi_ix,b,mh -> 8)
            mr = sb.tile([128, 8], dt.float32)
            for hi_ix, ti in enumerate([1, 4]):
                for b in range(B):
                    for mh in range(2):
                        col = (ti * 2 + b) * 2 + mh
                        wcl = 56 + (hi_ix * 2 + b) * 2 + mh
                        mcol = (hi_ix * 2 + b) * 2 + mh
                        nc.vector.tensor_tensor(mr[:, mcol:mcol + 1], sc_t[:, col:col + 1], allr[:, wcl:wcl + 1], AT.mult)
                        nc.vector.tensor_tensor(mr[:, mcol:mcol + 1], mr[:, mcol:mcol + 1], tc_t[:, col:col + 1], AT.add)
            # attn: per (ti,b): hsig(relu(sw . gap_tib + sb))
            swc = sb.tile([128, 2], dt.float32, tag='swc', name='swc')
            dma(swc[:], P['swcol'][:])
            swb = sb.tile([14, 1], dt.float32, tag='swb', name='swb')
            dma(swb[:], P['swb'][:])
            pat = ps.tile([14, 1], dt.float32, tag='psmall', name='pat')
            nc.tensor.matmul(pat[:], gap[:, 0:28:2], swc[:, 0:1], start=True, stop=False)
            nc.tensor.matmul(pat[:], gap[:, 1:28:2], swc[:, 1:2], start=False, stop=True)
            att = sb.tile([14, 1], dt.float32)
            nc.scalar.activation(att[:], pat[:], AF.Copy)
            nc.vector.tensor_tensor(att[:], att[:], swb[:], AT.add)
            tss(att[:], att[:], 0.0, AT.max)
            tss(att[:], att[:], 3.0, AT.add, 1.0 / 6.0, AT.mult)
            tss(att[:], att[:], 0.0, AT.max)
            tss(att[:], att[:], 1.0, AT.min)
            ones14 = sb.tile([14, 128], dt.float32)
            dma(ones14[:], P['ones14'][:])
            dI = sb.tile([14, 14], dt.float32)
            nc.vector.tensor_scalar(dI[:], ident[0:14, 0:14], att[0:14, :], None, AT.mult)
            pA = ps.tile([128, 14], dt.float32, tag='psmall', name='pA')
            nc.tensor.matmul(pA[:], ones14[:], dI[:], start=True, stop=True)
            attx_all = sb.tile([128, 14], dt.float32)
            nc.scalar.activation(attx_all[:], pA[:], AF.Copy)
            attx = {}
            for ti in range(7):
                for b in range(B):
                    attx[(ti, b)] = attx_all[:, ti * 2 + b:ti * 2 + b + 1]
            # dyrelu coefs per (l, b)
            dy1t = sb.tile([128, 128], dt.float32, tag='dy1t', name='dy1t')
            dma(dy1t[:], P['dy1t'][:])
            dy1b = sb.tile([64, 1], dt.float32, tag='dy1b', name='dy1b')
            dma(dy1b[:], P['dy1b'][:])
            dy2t = sb.tile([64, 1024], dt.float32, tag='dy2t', name='dy2t')
            dma(dy2t[:], P['dy2t'][:])
            dy2b = sb.tile([128, 8], dt.float32, tag='dy2b', name='dy2b')
            dma(dy2b[:], P['dy2b'][:])
            AB = {}
            for l in range(3):
                tis = cfg.ltasks[l]
                nb = float(len(tis))
                for b in range(B):
                    go = sb2.tile([128, 2], dt.float32, tag='go', name='go')
                    nc.vector.memset(go[:], 0.0)
                    for ti in tis:
                        t = cfg.tinfo[ti]
                        for mh in range(2):
                            col = (ti * 2 + b) * 2 + mh
                            src = gap[:, col:col + 1]
                            if t['br'] == 'hi':
                                hi_ix = [1, 4].index(ti)
                                src = mr[:, (hi_ix * 2 + b) * 2 + mh:(hi_ix * 2 + b) * 2 + mh + 1]
                            hsc = sb2.tile([128, 1], dt.float32, tag='hsc', name='hsc')
                            nc.vector.tensor_scalar(hsc[:], src, attx[(ti, b)], None, AT.mult)
                            nc.vector.tensor_tensor(go[:, mh:mh + 1], go[:, mh:mh + 1], hsc[:], AT.add)
                    tss(go[:], go[:], 1.0 / nb, AT.mult)
                    ph = ps.tile([64, 1], dt.float32, tag='psmall', name='ph')
                    nc.tensor.matmul(ph[:], dy1t[:, 0:64], go[:, 0:1], start=True, stop=False)
                    nc.tensor.matmul(ph[:], dy1t[:, 64:128], go[:, 1:2], start=False, stop=True)
                    h = sb2.tile([64, 1], dt.float32, tag='h', name='h')
                    nc.scalar.activation(h[:], ph[:], AF.Copy)
                    nc.vector.tensor_tensor(h[:], h[:], dy1b[:], AT.add)
                    tss(h[:], h[:], 0.0, AT.max)
                    pc8 = ps.tile([128, 8], dt.float32, tag='psmall', name='pc8')
                    for mt in range(8):
                        nc.tensor.matmul(pc8[:, mt:mt + 1], dy2t[:, mt * 128:(mt + 1) * 128], h[:], start=True, stop=True)
                    cf8 = sb2.tile([128, 8], dt.float32, tag='cf8', name='cf8')
                    nc.scalar.activation(cf8[:], pc8[:], AF.Copy)
                    nc.vector.tensor_tensor(cf8[:], cf8[:], dy2b[:], AT.add)
                    tss(cf8[:], cf8[:], 3.0, AT.add, 1.0 / 6.0, AT.mult)
                    tss(cf8[:], cf8[:], 0.0, AT.max)
                    tss(cf8[:], cf8[:], 1.0, AT.min)
                    ab = sb.tile([128, 8], dt.float32, tag=f'ab{l}{b}', name=f'ab{l}{b}')
                    tss(ab[:, 0:2], cf8[:, 0:2], -0.5, AT.add, 2.0, AT.mult)
                    tss(ab[:, 0:2], ab[:, 0:2], 1.0, AT.add)
                    tss(ab[:, 2:4], cf8[:, 2:4], -0.5, AT.add)
                    tss(ab[:, 4:6], cf8[:, 4:6], -0.5, AT.add, 2.0, AT.mult)
                    tss(ab[:, 6:8], cf8[:, 6:8], -0.5, AT.add)
                    AB[(l, b)] = ab

            if STAGE < 5:
                raise _StopBuild()
            # ========= PHASE G: GN apply, resize, mean, dyrelu, out ========
            for l in range(3):
                tis = cfg.ltasks[l]
                NpadM = cfg.tinfo[tis[0]]['Npad']
                for b in range(B):
                    macc = {}
                    for mh in range(2):
                        mt = sb.tile([128, NpadM], dt.float32, tag=f'macc{mh}', name=f'macc{mh}')
                        nc.vector.memset(mt[:], 0.0)
                        macc[mh] = mt
                    for ti in tis:
                        t = cfg.tinfo[ti]
                        col0 = (ti * 2 + b) * 2
                        if t['br'] != 'hi':
                            for mh in range(2):
                                ld = sb2.tile([128, NpadM], dt.float32, tag='ld', name='ld')
                                dma(ld[:], dcnout[(ti, b)][mh * 128:(mh + 1) * 128, :])
                                nc.vector.tensor_scalar(ld[:], ld[:], sc_t[:, col0 + mh:col0 + mh + 1],
                                                        tc_t[:, col0 + mh:col0 + mh + 1], AT.mult, AT.add)
                                nc.vector.tensor_scalar(ld[:], ld[:], attx[(ti, b)], None, AT.mult)
                                nc.vector.tensor_tensor(macc[mh][:], macc[mh][:], ld[:], AT.add)
                        else:
                            # GN -> channel-last dram bounce -> static resize gather
                            rz = cfg.resize[l]
                            npch_c = t['Npad'] // 128
                            hcl = dram.tile([t['Npad'], 256], dt.bfloat16, tag=f'hcl{l}{b}')
                            for mh in range(2):
                                ld = sb2.tile([128, t['Npad']], dt.float32, tag='ld')
                                dma(ld[:], dcnout[(ti, b)][mh * 128:(mh + 1) * 128, :])
                                nc.vector.tensor_scalar(ld[:], ld[:], sc_t[:, col0 + mh:col0 + mh + 1],
                                                        tc_t[:, col0 + mh:col0 + mh + 1], AT.mult, AT.add)
                                ldb = sb2.tile([128, t['Npad']], dt.bfloat16, tag='ldb')
                                nc.vector.tensor_copy(ldb[:], ld[:])
                                for pc in range(npch_c):
                                    ptr = ps.tile([128, 128], dt.bfloat16, tag='ptile', name='ptr')
                                    nc.tensor.transpose(ptr[:], ldb[:, pc * 128:(pc + 1) * 128], identb[:])
                                    trs = sb2.tile([128, 128], dt.bfloat16, tag='trs', name='trs')
                                    nc.vector.tensor_copy(trs[:], ptr[:])
                                    dma(hcl[pc * 128:(pc + 1) * 128, mh * 128:(mh + 1) * 128], trs[:])
                            npch_f = rz['Npad'] // 128
                            rzi = sb.tile([128, npch_f * 32], dt.int16, tag=f'rzi{l}', name=f'rzi{l}')
                            dma(rzi[:], P[f'rzidx{l}'][:])
                            rzc = sb.tile([128, npch_f * 4], dt.float32, tag=f'rzc{l}', name=f'rzc{l}')
                            dma(rzc[:], P[f'rzcoef{l}'][:])
                            nh = (npch_f + 1) // 2
                            Gz = sb2.tile([128, 36, 256], dt.bfloat16, tag='G', name='Gz')
                            Gz2 = sb2.tile([128, 36, 256], dt.bfloat16, tag='G', name='Gz2')
                            tot = npch_f * 512
                            off = 0
                            while off < tot:
                                nn = min(512, tot - off)
                                dst = Gz if off < nh * 512 else Gz2
                                dof = off if off < nh * 512 else off - nh * 512
                                nc.gpsimd.dma_gather(
                                    dst[:, dof // 128:(dof + nn) // 128, :], hcl[:],
                                    rzi[:, off // 16:(off + nn) // 16],
                                    num_idxs=nn, num_idxs_reg=nn,
                                    elem_size=256, queue_num=qn[0] % 4)
                                qn[0] += 1
                                off += nn
                            for pc in range(npch_f):
                                t0 = sb2.tile([128, 256], dt.bfloat16, tag='t0', name='t0')
                                t1 = sb2.tile([128, 256], dt.bfloat16, tag='t1', name='t1')
                                Gzc = Gz if pc < nh else Gz2
                                pcl = pc if pc < nh else pc - nh
                                nc.vector.tensor_scalar(t0[:], Gzc[:, pcl * 4 + 0, :], rzc[:, pc * 4:pc * 4 + 1], None, AT.mult)
                                nc.vector.tensor_scalar(t1[:], Gzc[:, pcl * 4 + 1, :], rzc[:, pc * 4 + 1:pc * 4 + 2], None, AT.mult)
                                nc.vector.tensor_tensor(t0[:], t0[:], t1[:], AT.add)
                                nc.vector.tensor_scalar(t1[:], Gzc[:, pcl * 4 + 2, :], rzc[:, pc * 4 + 2:pc * 4 + 3], None, AT.mult)
                                nc.vector.tensor_tensor(t0[:], t0[:], t1[:], AT.add)
                                nc.vector.tensor_scalar(t1[:], Gzc[:, pcl * 4 + 3, :], rzc[:, pc * 4 + 3:pc * 4 + 4], None, AT.mult)
                                nc.vector.tensor_tensor(t0[:], t0[:], t1[:], AT.add)
                                for mh in range(2):
                                    ptz = ps.tile([128, 128], dt.bfloat16, tag='ptile', name='ptz')
                                    nc.tensor.transpose(ptz[:], t0[:, mh * 128:(mh + 1) * 128], identb[:])
                                    rzs = sb2.tile([128, 128], dt.float32, tag='rzs', name='rzs')
                                    nc.vector.tensor_scalar(rzs[:], ptz[:], attx[(ti, b)], None, AT.mult)
                                    nc.vector.tensor_tensor(macc[mh][:, pc * 128:(pc + 1) * 128],
                                                            macc[mh][:, pc * 128:(pc + 1) * 128], rzs[:], AT.add)
                    nbi = 1.0 / len(tis)
                    ab = AB[(l, b)]
                    for mh in range(2):
                        tss(macc[mh][:], macc[mh][:], nbi, AT.mult)
                        o1 = sb.tile([128, NpadM], dt.float32, tag='o1', name='o1')
                        nc.vector.tensor_scalar(o1[:], macc[mh][:], ab[:, 0 + mh:1 + mh],
                                                ab[:, 2 + mh:3 + mh], AT.mult, AT.add)
                        o2 = sb2.tile([128, NpadM], dt.float32, tag='ld', name='o2')
                        nc.vector.tensor_scalar(o2[:], macc[mh][:], ab[:, 4 + mh:5 + mh],
                                                ab[:, 6 + mh:7 + mh], AT.mult, AT.add)
                        nc.vector.tensor_tensor(o1[:], o1[:], o2[:], AT.max)
                        NW = cfg.fine[l]['span'] * HW_L[l][1]
                        dma(OUT[l][b, mh * 128:(mh + 1) * 128, :], o1[:, 0:NW])
    nc.compile()
    return nc


# ===========================================================================
# entry point
# ===========================================================================

def kernel(**inputs):
    import sys
    if '/opt/trn_rl_repo' not in sys.path:
        sys.path.insert(0, '/opt/trn_rl_repo')
    from concourse.bass_utils import run_bass_kernel_spmd
    cfg = CFG
    in_maps = host_prep(inputs, cfg)
    if 'nc' not in _CACHE:
        _CACHE['nc'] = build_nc(cfg)
    nc = _CACHE['nc']
    res = run_bass_kernel_spmd(nc, in_maps, core_ids=list(range(NCOR)))
    outs = []
    for l in range(3):
        H, W = HW_L[l]
        full = np.zeros((B, C, H, W), np.float32)
        f = cfg.fine[l]
        for c in range(NCOR):
            o = res.results[c][f'out{l}'].reshape(B, C, f['span'], W)
            s, e, w0 = f['s'][c], f['e'][c], f['win'][c]
            full[:, :, s:e, :] = o[:, :, s - w0:e - w0, :]
        outs.append(full)
    return tuple(outs)
